# revision 2
# baseline (speedup 1.0000x reference)
"""Trainium2 Bass kernel for BiDirectionalFusionBlock.

Data-parallel over batch: B=32 -> 8 cores x 4 local batch.
Per core, per refine iteration (R=3), per local batch element:
  1. rasterize: d2 via split-bf16 PE matmul -> ACT exp -> splat w [T,J];
     (wsum, vx, vy) via PE matmul over t; heat via DVE max +
     gpsimd partition_all_reduce.
  2. in_proj as 1x1-conv matmul (K=67: Ms 64ch + P 3ch; t_embed folded
     into a per-(b,out-ch) bias; 2x2 avg-pool of M folded into weights).
  3. 3 residual blocks: conv3x3 = 5 fp8e4m3 DoubleRow pair-matmuls (2
     taps per PE pass) on a 66-pitch zero-padded layout; weights
     pre-scaled by 32 into fp8 normal range (GroupNorm downstream is
     scale-invariant); GroupNorm via bn_stats/bn_aggr +
     indicator-matmul group reduce; SiLU fused into ACT affine pass.
  4. grid_sample: gpsimd ap_gather of 4 bilinear corners (zero padding
     free from the padded layout), out_proj + MLP on the 256 gathered
     points; x <- clip(x + 0.2*dx).

Two batch elements are software-pipelined (emission interleaved via
generators) so one batch's PE matmul stream fills the other's
GroupNorm/SiLU dependency stalls. Cross-engine row moves ([1,T] ->
rows of [10,TB]) go through one-hot PE matmuls instead of per-row
DMAs; the grid-sample index/weight DRAM bounces are single batched
DMAs; long SiLU/residual passes are chunked so consumers can start
early.
"""

from contextlib import ExitStack

import numpy as np

import concourse.bass as bass
import concourse.bacc as bacc
import concourse.tile as tile
from concourse import bass_isa, library_config, mybir
from concourse.bass_utils import run_bass_kernel_spmd

F32 = mybir.dt.float32
BF16 = mybir.dt.bfloat16
FP8 = mybir.dt.float8e4
I32 = mybir.dt.int32
U16 = mybir.dt.uint16
I16 = mybir.dt.int16
AF = mybir.ActivationFunctionType
ALU = mybir.AluOpType
PM_DR = mybir.MatmulPerfMode.DoubleRow

# conv weights are scaled up by WSCALE before the fp8e4m3 cast (raw std
# 0.02 sits in the subnormal range); GroupNorm right after each conv is
# scale-invariant so nothing needs un-scaling (conv bias is scaled too).
# 16 (not 32): raw conv outputs must stay below fp8e4m3 inf (|x|>255).
WSCALE = 16.0

# problem constants (hardcoded; kernel must be self-contained)
NCORES = 8
B_FULL = 32
BL = B_FULL // NCORES      # local batch = 4
T = 256
TB = BL * T                # 1024
CM, CP, CT, CS, HID = 64, 3, 128, 128, 128
CIN = CM + CP + CT         # 195
HS = WS = 64
J = HS * WS                # 4096
HIN = WIN = 128
R = 3
NB = 3
GROUPS = 8
GSZ = CS // GROUPS         # 16
EPS_GN = 1e-5
SIGMA = 1.2
UPD = 0.2
A_EXP = float(np.float32(-0.5) / np.float32(SIGMA * SIGMA + 1e-8))

# padded conv layout: 66-pitch rows + pad ring, stored at +1 (PADOFF) so
# corner taps of the first/last chunk stay in-bounds. p=(y+1)*66+(x+1)+1
PW = WS + 2                # 66
NPAD = PW * (HS + 2)       # 4356
PADOFF = 1
NTILE = NPAD + 2           # 4358
KROWS = 10                 # split-bf16 d2 matmul contraction rows
TAPOFF = [-PW - 1, -PW, -PW + 1, -1, 0, 1, PW - 1, PW, PW + 1]


def emit(ctx: ExitStack, tc: tile.TileContext, io: dict):
    nc = tc.nc

    singles = ctx.enter_context(tc.tile_pool(name="singles", bufs=1))
    psmall = ctx.enter_context(tc.tile_pool(name="psmall", bufs=2, space="PSUM"))

    def valid3(t, row0, nrows):
        """[128, nrows, 64] view of valid cells, padded rows row0..row0+nrows"""
        base = (row0 + 1) * PW + PADOFF
        sl = t[:, base : base + nrows * PW]
        return sl.rearrange("p (r c) -> p r c", c=PW)[:, :, 1 : 1 + WS]

    def zero_pads(t):
        nc.vector.memset(t[:, 0 : PADOFF + PW + 1], 0.0)
        nc.vector.memset(t[:, PADOFF + NPAD - PW : NTILE], 0.0)
        ring = t[:, PADOFF + PW : PADOFF + PW + 64 * PW].rearrange(
            "p (r c) -> p r c", c=PW
        )
        nc.vector.memset(ring[:, :, 0:1], 0.0)
        nc.vector.memset(ring[:, :, 65:66], 0.0)

    # =========== setup (stage/mload pools freed before main loop) ========
    # conv weights as fp8 DoubleRow pairs: pair p holds taps (2p, 2p+1),
    # pair 4 holds tap 8 + a zero plane. Tile [128, 256] = [i, (two o)].
    conv_pair = [[[None] * 5 for _ in range(2)] for _ in range(NB)]
    with ExitStack() as sctx:
        stage = sctx.enter_context(tc.tile_pool(name="stage", bufs=2))
        mload = sctx.enter_context(tc.tile_pool(name="mload", bufs=2))

        # identity matrices
        iden_i = stage.tile([128, 128], I32, tag="ideni")
        nc.gpsimd.iota(iden_i[:, :], pattern=[[1, 128]], base=0,
                       channel_multiplier=-1)
        iden_f = stage.tile([128, 128], F32, tag="idenf")
        nc.vector.tensor_copy(iden_f[:, :], iden_i[:, :])
        id32 = singles.tile([128, 128], F32)
        nc.vector.tensor_scalar(id32[:, :], iden_f[:, :], 0.0, None, ALU.is_equal)
        idbf = singles.tile([128, 128], BF16)
        nc.vector.tensor_copy(idbf[:, :], id32[:, :])

        # one-hot row-selector for PE row placement: window r = cols
        # [9r, 9r+8) holds e_r (a 1 at col 9r+r)
        sel = singles.tile([1, 128], BF16)
        nc.vector.memset(sel[:, :], 0.0)
        for r in range(8):
            nc.vector.memset(sel[:1, 9 * r + r : 9 * r + r + 1], 1.0)

        # conv weights: [o,(i 3 3)] -> fp8 pair lhsT tiles [i, (two, o)],
        # scaled by WSCALE (GN downstream is scale-invariant)
        for blk in range(NB):
            for cv, nm in enumerate(("c1w", "c2w")):
                st = stage.tile([128, 1152], F32, tag="wstage")
                nc.sync.dma_start(out=st[:, :], in_=io[nm][blk])
                stv = st[:, :].rearrange("p (i n) -> p i n", n=9)
                for p in range(5):
                    lt = singles.tile([128, 256], FP8, tag=f"cp{blk}{cv}{p}")
                    for h in range(2):
                        k = 2 * p + h
                        if k < 9:
                            pt = psmall.tile([128, 256], F32, tag="ps")
                            nc.tensor.matmul(
                                pt[:, :128], stv[:, :, k], id32[:, :],
                                is_transpose=True,
                            )
                            nc.scalar.activation(
                                lt[:, h * 128 : (h + 1) * 128], pt[:, :128],
                                AF.Identity, scale=WSCALE,
                            )
                        else:
                            nc.vector.memset(
                                lt[:, h * 128 : (h + 1) * 128], 0.0
                            )
                    conv_pair[blk][cv][p] = lt

        # in_proj W^T
        ipst = stage.tile([128, 195], F32, tag="ipst")
        nc.sync.dma_start(out=ipst[:, :], in_=io["ipw"][:, :])
        WtA = singles.tile([128, 128], FP8)      # rows i=0..127
        WtA32 = stage.tile([128, 128], F32, tag="wta32")
        WtB32 = stage.tile([67, 128], F32, tag="wtb32")
        ptA = psmall.tile([128, 256], F32, tag="ps")
        nc.tensor.matmul(ptA[:, :128], ipst[:, 0:128], id32[:, :],
                         is_transpose=True)
        nc.vector.tensor_copy(WtA32[:, :], ptA[:, :128])
        ptB = psmall.tile([128, 256], F32, tag="ps")
        nc.tensor.matmul(ptB[:67, :128], ipst[:, 128:195], id32[:, :],
                         is_transpose=True)
        nc.vector.tensor_copy(WtB32[:, :], ptB[:67, :128])
        # fp8 copy; fold 2x2 avg-pool 0.25 into Ms rows and WSCALE into
        # everything (un-scaled by the in_proj ACT's 1/WSCALE)
        nc.vector.tensor_scalar(WtA[0:64, :], WtA32[0:64, :], 0.25 * WSCALE,
                                None, ALU.mult)
        nc.vector.tensor_scalar(WtA[64:128, :], WtA32[64:128, :], WSCALE,
                                None, ALU.mult)

        # t_embed^T
        test_ = stage.tile([4, 128], F32, tag="test")
        nc.sync.dma_start(out=test_[:, :], in_=io["tE"][:, :])
        tEt = singles.tile([128, 4], F32)
        ptT = psmall.tile([128, 256], F32, tag="ps")
        nc.tensor.matmul(ptT[:, :4], test_[:, :], id32[:4, :4],
                         is_transpose=True)
        nc.vector.tensor_copy(tEt[:, :], ptT[:, :4])

        def col_from_vec(dram_ap, n, nm):
            row = stage.tile([1, 128], F32, tag="rowst")
            nc.sync.dma_start(out=row[:1, :n], in_=dram_ap.unsqueeze(0))
            col = singles.tile([n, 1], F32, tag=f"col_{nm}")
            pt = psmall.tile([128, 256], F32, tag="ps")
            nc.tensor.matmul(pt[:n, :1], row[:1, :n], id32[:1, :1],
                             is_transpose=True)
            nc.vector.tensor_copy(col[:, :], pt[:n, :1])
            return col

        ipb_c = col_from_vec(io["ipb"], 128, "ipb")
        ob_c = col_from_vec(io["ob"], 128, "ob")
        b1_c = col_from_vec(io["b1"], 128, "b1")
        b2_c = col_from_vec(io["b2"], 128, "b2")
        # b3 split per coordinate
        row3 = stage.tile([1, 128], F32, tag="rowst")
        nc.sync.dma_start(out=row3[:1, :2], in_=io["b3"].unsqueeze(0))
        b3x = singles.tile([1, 1], F32)
        nc.vector.tensor_copy(b3x[:, :], row3[0:1, 0:1])
        b3y = singles.tile([1, 1], F32)
        nc.vector.tensor_copy(b3y[:, :], row3[0:1, 1:2])
        b3c2 = {"x": b3x[:, :], "y": b3y[:, :]}

        # bias_S[o,b] = ipb + W^T[67:195]·tE_b   (t_map contribution)
        Wt_t = stage.tile([128, 128], F32, tag="wtt")
        ptW = psmall.tile([128, 256], F32, tag="ps")
        nc.tensor.matmul(ptW[:, :128], ipst[:, 67:195], id32[:, :],
                         is_transpose=True)
        nc.vector.tensor_copy(Wt_t[:, :], ptW[:, :128])
        pbs = psmall.tile([128, 256], F32, tag="ps")
        nc.tensor.matmul(pbs[:, :4], Wt_t[:, :], tEt[:, :])
        bias_S = singles.tile([128, 4], F32)
        nc.vector.tensor_scalar(bias_S[:, :], pbs[:, :4], ipb_c[:, :], None,
                                ALU.add)

        # mlp weights (memory layout is already lhsT)
        w1a32 = stage.tile([128, 128], F32, tag="w1a32")
        nc.sync.dma_start(out=w1a32[:, :], in_=io["w1"][0:128, :])
        w1b32 = stage.tile([128, 128], F32, tag="w1b32")
        nc.sync.dma_start(out=w1b32[:, :], in_=io["w1"][128:256, :])
        w1a = singles.tile([128, 128], BF16)
        nc.vector.tensor_copy(w1a[:, :], w1a32[:, :])
        w2st = stage.tile([128, 128], F32, tag="w2st")
        nc.sync.dma_start(out=w2st[:, :], in_=io["w2"][:, :])
        w2b = singles.tile([128, 128], BF16)
        nc.vector.tensor_copy(w2b[:, :], w2st[:, :])
        w3st = stage.tile([128, 2], F32, tag="w3st")
        nc.sync.dma_start(out=w3st[:, :], in_=io["w3"][:, :])
        w3b = singles.tile([128, 2], BF16)
        nc.vector.tensor_copy(w3b[:, :], w3st[:, :])
        owst = stage.tile([128, 128], F32, tag="owst")
        nc.sync.dma_start(out=owst[:, :], in_=io["ow"][:, :])
        owT = singles.tile([128, 128], BF16)
        pow_ = psmall.tile([128, 256], F32, tag="ps")
        nc.tensor.matmul(pow_[:, :128], owst[:, :], id32[:, :],
                         is_transpose=True)
        nc.scalar.copy(owT[:, :], pow_[:, :128])

        # tb1[h,b] = b1 + W1[128:256]^T tE_b + W1[0:128]^T out_b
        ob4 = singles.tile([128, 4], F32)
        for c in range(4):
            nc.vector.tensor_copy(ob4[:, c : c + 1], ob_c[:, :])
        ptb = psmall.tile([128, 256], F32, tag="ps")
        nc.tensor.matmul(ptb[:, :4], w1b32[:, :], tEt[:, :], start=True,
                         stop=False)
        nc.tensor.matmul(ptb[:, :4], w1a32[:, :], ob4[:, :], start=False,
                         stop=True)
        tb1 = singles.tile([128, 4], F32)
        nc.vector.tensor_scalar(tb1[:, :], ptb[:, :4], b1_c[:, :], None,
                                ALU.add)

        # gamma/beta/conv-bias columns: [3,128] dram -> [128,3] f32
        gcols = {}
        for nm in ("c1b", "g1w", "g1b", "c2b", "g2w", "g2b"):
            st = stage.tile([3, 128], F32, tag="gst")
            nc.sync.dma_start(out=st[:, :], in_=io[nm][:, :])
            col = singles.tile([128, 3], F32, tag=f"gc{nm}")
            pt = psmall.tile([128, 256], F32, tag="ps")
            nc.tensor.matmul(pt[:, :3], st[:, :], id32[:3, :3],
                             is_transpose=True)
            if nm in ("c1b", "c2b"):
                # conv bias rides the WSCALE'd conv output into GN
                nc.vector.tensor_scalar(col[:, :], pt[:, :3], WSCALE, None,
                                        ALU.mult)
            else:
                nc.vector.tensor_copy(col[:, :], pt[:, :3])
            gcols[nm] = col

        eps8 = singles.tile([8, 1], F32)
        nc.vector.memset(eps8[:, :], EPS_GN)
        # GN group-reduce indicator (iota: p in [16g, 16g+16)), with the
        # padded-column count scale folded in
        ind16 = singles.tile([128, 8], F32)
        ii1 = stage.tile([128, 8], I32, tag="ii1")
        nc.gpsimd.iota(ii1[:, :], pattern=[[-16, 8]], base=0,
                       channel_multiplier=1)
        if1 = stage.tile([128, 8], F32, tag="if1")
        nc.vector.tensor_scalar(if1[:, :], ii1[:, :], 0, None, ALU.is_ge)
        if2 = stage.tile([128, 8], F32, tag="if2")
        nc.vector.tensor_scalar(if2[:, :], ii1[:, :], 16,
                                (NTILE / 4096.0) / GSZ, ALU.is_lt, ALU.mult)
        nc.vector.tensor_tensor(ind16[:, :], if1[:, :], if2[:, :], ALU.mult)

        # rasterize static rhs rows [10, J] bf16, built in [32,128] layout:
        # k:      0     1     2     3     4     5     6    7    8    9
        # lhsT:   xh    yh    xh    yh    xl    yl    sqh  sql  1    1
        # rhs:    sxh   syh   sxl   syl   sxh   syh   1    1    ssh  ssl
        #   sx = -2*xs (split h+l), ss = xs^2+ys^2 (split h+l)
        rhs_r = singles.tile([KROWS, J], BF16)
        jx32 = stage.tile([32, 128], I32, tag="jx32")
        nc.gpsimd.iota(jx32[:, :], pattern=[[0, 2], [1, 64]], base=0,
                       channel_multiplier=0)
        jy32 = stage.tile([32, 128], I32, tag="jy32")
        nc.gpsimd.iota(jy32[:, :], pattern=[[1, 2], [0, 64]], base=0,
                       channel_multiplier=2)

        def split_rows(val32, rh, rl, scale):
            """val32 [32,128] f32 * scale -> bf16 h+l, DMA'd to rhs_r rows"""
            m2 = stage.tile([32, 128], F32, tag="spl_m2")
            nc.vector.tensor_scalar(m2[:, :], val32[:, :], scale, None, ALU.mult)
            hh = stage.tile([32, 128], BF16, tag="spl_h")
            nc.vector.tensor_copy(hh[:, :], m2[:, :])
            rr = stage.tile([32, 128], F32, tag="spl_r")
            nc.vector.tensor_tensor(rr[:, :], m2[:, :], hh[:, :], ALU.subtract)
            ll = stage.tile([32, 128], BF16, tag="spl_l")
            nc.vector.tensor_copy(ll[:, :], rr[:, :])
            for r, tl in ((rh, hh), (rl, ll)):
                nc.sync.dma_start(out=rhs_r[r : r + 1, :], in_=tl[:, :])

        jxf = stage.tile([32, 128], F32, tag="jxf")
        nc.vector.tensor_copy(jxf[:, :], jx32[:, :])
        jyf = stage.tile([32, 128], F32, tag="jyf")
        nc.vector.tensor_copy(jyf[:, :], jy32[:, :])
        split_rows(jxf, 0, 2, -2.0)
        split_rows(jyf, 1, 3, -2.0)
        nc.sync.dma_start(out=rhs_r[4:5, :], in_=rhs_r[0:1, :])
        nc.sync.dma_start(out=rhs_r[5:6, :], in_=rhs_r[1:2, :])
        ones_bf = singles.tile([32, 128], BF16)
        nc.vector.memset(ones_bf[:, :], 1.0)
        nc.sync.dma_start(out=rhs_r[6:7, :], in_=ones_bf[:, :])
        nc.sync.dma_start(out=rhs_r[7:8, :], in_=ones_bf[:, :])
        ss32 = stage.tile([32, 128], F32, tag="ss32")
        nc.vector.tensor_tensor(ss32[:, :], jxf[:, :], jxf[:, :], ALU.mult)
        sy32 = stage.tile([32, 128], F32, tag="sy32")
        nc.vector.tensor_tensor(sy32[:, :], jyf[:, :], jyf[:, :], ALU.mult)
        nc.vector.tensor_tensor(ss32[:, :], ss32[:, :], sy32[:, :], ALU.add)
        split_rows(ss32, 8, 9, 1.0)

        # x0 [TB,2] -> x_x / x_y [1,TB] f32 (separate so all ops are base-0)
        x_x = singles.tile([1, TB], F32)
        x_y = singles.tile([1, TB], F32)
        for c in range(8):
            xst = stage.tile([128, 2], F32, tag="xst")
            nc.sync.dma_start(out=xst[:, :],
                              in_=io["x0"][c * 128 : (c + 1) * 128, :])
            for d, xrow in ((0, x_x), (1, x_y)):
                pt = psmall.tile([128, 256], F32, tag="ps")
                nc.tensor.matmul(pt[:1, :128], xst[:, d : d + 1], id32[:, :],
                                 is_transpose=True)
                nc.vector.tensor_copy(xrow[:1, c * 128 : (c + 1) * 128],
                                      pt[0:1, :128])

        # M load + 2x2 pool (sum; 0.25 folded in WtA)
        inp_b = [
            singles.tile([67, J], FP8, tag=f"inp{b}", name=f"inp{b}")
            for b in range(BL)
        ]
        for bp in range(BL // 2):  # b-pairs stacked on 128 partitions
            for ch in range(8):
                mt = mload.tile([128, 2048], F32, tag="mt")
                src = io["M"][2 * bp : 2 * bp + 2, :,
                              ch * 2048 : (ch + 1) * 2048]
                nc.sync.dma_start(out=mt[:, :],
                                  in_=src.rearrange("b c f -> (b c) f"))
                a1 = mload.tile([128, 1024], F32, tag="a1")
                mv = mt[:, :].rearrange("p (y q x) -> p y q x", q=2, x=64)
                nc.vector.tensor_tensor(
                    a1[:, :].rearrange("p (y x) -> p y x", x=64),
                    mv[:, :, 0, :], mv[:, :, 1, :], ALU.add,
                )
                av = a1[:, :].rearrange("p (y q x) -> p y q x", q=2, x=64)
                for h in range(2):
                    b = 2 * bp + h
                    dst = inp_b[b][0:64, ch * 512 : (ch + 1) * 512]
                    nc.vector.tensor_tensor(
                        dst.rearrange("p (y x) -> p y x", x=64),
                        av[h * 64 : h * 64 + 64, :, 0, :],
                        av[h * 64 : h * 64 + 64, :, 1, :], ALU.add,
                    )

    # persistent pools (entered after setup scratch is released)
    wsplat = ctx.enter_context(tc.tile_pool(name="wsplat", bufs=2))
    spool = ctx.enter_context(tc.tile_pool(name="spool", bufs=2))
    hpool = ctx.enter_context(tc.tile_pool(name="hpool", bufs=2))
    statp = ctx.enter_context(tc.tile_pool(name="statp", bufs=2))
    ppool = ctx.enter_context(tc.tile_pool(name="ppool", bufs=1))
    gpool = ctx.enter_context(tc.tile_pool(name="gpool", bufs=2))
    sums32 = ctx.enter_context(tc.tile_pool(name="sums32", bufs=2))
    pbig = ctx.enter_context(tc.tile_pool(name="pbig", bufs=3, space="PSUM"))
    psums = ctx.enter_context(tc.tile_pool(name="psums", bufs=2, space="PSUM"))

    # dram bounce for the grid-sample index wrap ([R, BL, 4*T])
    qb = io["qb"]

    # dynamic raster lhsT [10, TB]; rows 8,9 = ones
    lhsT_r = singles.tile([KROWS, TB], BF16)
    nc.sync.dma_start(out=lhsT_r[8:9, :], in_=ones_bf[:8, :])
    nc.sync.dma_start(out=lhsT_r[9:10, :], in_=ones_bf[:8, :])

    # ---------------- per-(it,b) step as a generator ----------------
    # GN: one PE matmul for the group sums, then a replicating DMA
    # ([8,2] -> [128,2]) instead of a second PE matmul, so the conv
    # stream behind it in the PE queue never waits on the DVE round-trip.
    def gn_coeffs(stat6, gamma, beta, slot):
        mv_ = statp.tile([128, 2], F32, tag=f"mv{slot}", name="mv_")
        nc.vector.bn_aggr(mv_[:, :], stat6[:, :, :])
        st2 = statp.tile([128, 2], F32, tag=f"st2{slot}", name="st2")
        nc.vector.tensor_copy(st2[:, 0:1], mv_[:, 0:1])
        nc.vector.tensor_tensor(st2[:, 1:2], mv_[:, 0:1], mv_[:, 0:1],
                                ALU.mult)
        nc.vector.tensor_tensor(st2[:, 1:2], st2[:, 1:2], mv_[:, 1:2],
                                ALU.add)
        pg = psmall.tile([128, 256], F32, tag="ps")
        nc.tensor.matmul(pg[:8, :2], ind16[:, :], st2[:, :])
        g8 = statp.tile([8, 2], F32, tag=f"g8{slot}", name="g8")
        nc.vector.tensor_copy(g8[:, :], pg[:8, :2])
        g2 = statp.tile([8, 2], F32, tag=f"g2{slot}", name="g2")
        gmsq = statp.tile([8, 1], F32, tag=f"gmsq{slot}", name="gmsq")
        nc.vector.tensor_copy(g2[:, 0:1], g8[:, 0:1])
        nc.vector.tensor_tensor(gmsq[:, :], g8[:, 0:1], g8[:, 0:1],
                                ALU.mult)
        gvar = statp.tile([8, 1], F32, tag=f"gvar{slot}", name="gvar")
        nc.vector.tensor_tensor(gvar[:, :], g8[:, 1:2], gmsq[:, :],
                                ALU.subtract)
        gstd = statp.tile([8, 1], F32, tag=f"gstd{slot}", name="gstd")
        nc.scalar.activation(gstd[:, :], gvar[:, :], AF.Sqrt,
                             bias=eps8[:, :])
        nc.vector.reciprocal(g2[:, 1:2], gstd[:, :])
        ex = statp.tile([128, 2], F32, tag=f"ex{slot}", name="ex")
        g2s = g2[0:8, 0:2]
        nc.sync.dma_start(
            out=ex[:, :],
            in_=bass.AP(tensor=g2s.tensor, offset=g2s.offset,
                        ap=[[g2s.ap[0][0], 8], [0, GSZ], [1, 2]]),
        )
        sc = statp.tile([128, 1], F32, tag=f"sc{slot}", name="sc")
        nc.vector.tensor_tensor(sc[:, :], ex[:, 1:2], gamma, ALU.mult)
        bc = statp.tile([128, 1], F32, tag=f"bc{slot}", name="bc")
        nc.vector.tensor_tensor(bc[:, :], ex[:, 0:1], sc[:, :],
                                ALU.mult)
        nc.vector.tensor_tensor(bc[:, :], beta, bc[:, :], ALU.subtract)
        return sc, bc

    def conv(dst_pad, src_pad, pairs, bias_col, stat6):
        """conv3x3 via 5 fp8 DoubleRow pair-matmuls per 7-row chunk"""
        zero_pads(dst_pad)
        pitch = src_pad[:, 0:1].ap[0][0]
        for c in range(10):
            rc = 7 if c < 9 else 1
            base = (7 * c + 1) * PW + PADOFF
            n = rc * PW
            pc = pbig.tile([128, 512], F32, tag="pb")
            for p in range(5):
                ka = 2 * p
                off_a = base + TAPOFF[ka]
                delta = (TAPOFF[ka + 1] - TAPOFF[ka]) if p < 4 else 0
                s = src_pad[:, off_a : off_a + 1]
                rhs2 = bass.AP(
                    tensor=s.tensor, offset=s.offset,
                    ap=[[pitch, 128], [delta, 2], [1, n]],
                )
                nc.tensor.matmul(
                    pc[:, :n],
                    pairs[p][:, :].rearrange("q (two m) -> q two m", two=2),
                    rhs2,
                    start=(p == 0), stop=(p == 4),
                    perf_mode=PM_DR,
                )
            vout = valid3(dst_pad, 7 * c, rc)
            vin = pc[:, :n].rearrange("p (r c) -> p r c", c=PW)[
                :, :, 1 : 1 + WS
            ]
            nc.scalar.activation(vout, vin, AF.Identity, bias=bias_col)
        # stats over full padded rows (pads are zero; count scale is
        # folded into ind16)
        for c in range(9):
            lo = c * 512
            hi = min(NTILE, lo + 512)
            nc.vector.bn_stats(stat6[:, c, :], dst_pad[:, lo:hi])

    def step(it, b, slot):
        bt = b * T
        xs_x = x_x[:1, bt : bt + T]
        xs_y = x_y[:1, bt : bt + T]
        # ---- point prep (per b, [1, T] base-0 tiles) ----
        px = ppool.tile([1, T], F32, tag="px", name="px")
        nc.vector.tensor_scalar(px[:1, :], xs_x, 31.5, 31.5, ALU.mult,
                                ALU.add)
        py = ppool.tile([1, T], F32, tag="py", name="py")
        nc.vector.tensor_scalar(py[:1, :], xs_y, 31.5, 31.5, ALU.mult,
                                ALU.add)
        vx = ppool.tile([1, T], F32, tag=f"vx{slot}", name="vx")
        nc.vector.tensor_tensor(vx[:1, 1:T], px[:1, 1:T], px[:1, 0 : T - 1],
                                ALU.subtract)
        nc.vector.memset(vx[:1, 0:1], 0.0)
        vy = ppool.tile([1, T], F32, tag=f"vy{slot}", name="vy")
        nc.vector.tensor_tensor(vy[:1, 1:T], py[:1, 1:T], py[:1, 0 : T - 1],
                                ALU.subtract)
        nc.vector.memset(vy[:1, 0:1], 0.0)
        # lsums[h][t,0:2] = (vx,vy) transposed; col 2 = ones
        lsums = []
        for h in range(2):
            ls = sums32.tile([128, 3], BF16, tag=f"ls{h}{slot}",
                             name=f"ls{h}")
            for d, src in ((0, vx), (1, vy)):
                pt = psmall.tile([128, 256], F32, tag="ps")
                nc.tensor.matmul(pt[:128, :1],
                                 src[:1, h * 128 : h * 128 + 128],
                                 id32[:1, :1], is_transpose=True)
                nc.vector.tensor_copy(ls[:, d : d + 1], pt[:128, 0:1])
            nc.vector.memset(ls[:, 2:3], 1.0)
            lsums.append(ls)
        # split-bf16 rows of lhsT_r, placed via one-hot PE matmuls
        xh = ppool.tile([1, T], BF16, tag=f"xh{slot}", name="xh")
        nc.vector.tensor_copy(xh[:1, :], px[:1, :])
        yh = ppool.tile([1, T], BF16, tag=f"yh{slot}", name="yh")
        nc.vector.tensor_copy(yh[:1, :], py[:1, :])
        t1 = ppool.tile([1, T], F32, tag="t1", name="t1")
        nc.vector.tensor_tensor(t1[:1, :], px[:1, :], xh[:1, :],
                                ALU.subtract)
        xl = ppool.tile([1, T], BF16, tag=f"xl{slot}", name="xl")
        nc.vector.tensor_copy(xl[:1, :], t1[:1, :])
        t2 = ppool.tile([1, T], F32, tag="t2", name="t2")
        nc.vector.tensor_tensor(t2[:1, :], py[:1, :], yh[:1, :],
                                ALU.subtract)
        yl = ppool.tile([1, T], BF16, tag=f"yl{slot}", name="yl")
        nc.vector.tensor_copy(yl[:1, :], t2[:1, :])
        s1 = ppool.tile([1, T], F32, tag="s1", name="s1")
        nc.vector.tensor_tensor(s1[:1, :], px[:1, :], px[:1, :], ALU.mult)
        s2 = ppool.tile([1, T], F32, tag="s2", name="s2")
        nc.vector.tensor_tensor(s2[:1, :], py[:1, :], py[:1, :], ALU.mult)
        nc.vector.tensor_tensor(s1[:1, :], s1[:1, :], s2[:1, :], ALU.add)
        sqh = ppool.tile([1, T], BF16, tag=f"sqh{slot}", name="sqh")
        nc.vector.tensor_copy(sqh[:1, :], s1[:1, :])
        nc.vector.tensor_tensor(s2[:1, :], s1[:1, :], sqh[:1, :],
                                ALU.subtract)
        sql = ppool.tile([1, T], BF16, tag=f"sql{slot}", name="sql")
        nc.vector.tensor_copy(sql[:1, :], s2[:1, :])
        ps8 = psums.tile([8, 512], F32, tag="ps8", bufs=1, name="ps8")
        rowsrc = (xh, yh, xh, yh, xl, yl, sqh, sql)
        for r in range(8):
            nc.tensor.matmul(
                ps8[:8, :T], sel[0:1, 9 * r : 9 * r + 8], rowsrc[r][:1, :],
                start=(r == 0), stop=(r == 7),
            )
        nc.scalar.copy(lhsT_r[0:8, bt : bt + T], ps8[:8, :T])
        # grid-sample coords (floor + frac), per coordinate
        fr = {}
        om = {}
        fl = {}
        for cd, xs in (("x", xs_x), ("y", xs_y)):
            g_ = ppool.tile([1, T], F32, tag=f"g{cd}", name=f"g{cd}")
            nc.vector.tensor_scalar(g_[:1, :], xs, 32.0, 31.5, ALU.mult,
                                    ALU.add)
            xi = ppool.tile([1, T], I32, tag="xi", name="xi")
            nc.vector.tensor_copy(xi[:1, :], g_[:1, :])
            f_ = ppool.tile([1, T], F32, tag=f"f{cd}", name=f"f{cd}")
            nc.vector.tensor_copy(f_[:1, :], xi[:1, :])
            gt_ = ppool.tile([1, T], F32, tag="gt", name="gt")
            nc.vector.tensor_tensor(gt_[:1, :], f_[:1, :], g_[:1, :],
                                    ALU.is_gt)
            nc.vector.tensor_tensor(f_[:1, :], f_[:1, :], gt_[:1, :],
                                    ALU.subtract)
            nc.vector.tensor_tensor(g_[:1, :], g_[:1, :], f_[:1, :],
                                    ALU.subtract)
            o_ = ppool.tile([1, T], F32, tag=f"om{cd}", name=f"om{cd}")
            nc.vector.tensor_scalar(o_[:1, :], g_[:1, :], -1.0, 1.0,
                                    ALU.mult, ALU.add)
            fr[cd], om[cd], fl[cd] = g_, o_, f_
        q1 = ppool.tile([1, T], F32, tag="q1", name="q1")
        nc.vector.tensor_scalar(q1[:1, :], fl["y"][:1, :], 66.0,
                                float(67 + PADOFF), ALU.mult, ALU.add)
        q2 = ppool.tile([1, T], F32, tag="q2", name="q2")
        nc.vector.tensor_tensor(q2[:1, :], q1[:1, :], fl["x"][:1, :],
                                ALU.add)
        # 4 corner indices into one [1, 4T] row, one DMA to DRAM
        ci_all = ppool.tile([1, 4 * T], U16, tag="ci", name="ci_all")
        nc.vector.tensor_copy(ci_all[:1, 0:T], q2[:1, :])
        for k, off in ((1, 1), (2, 66), (3, 67)):
            nc.vector.tensor_scalar(ci_all[:1, k * T : (k + 1) * T],
                                    ci_all[:1, 0:T], off, None, ALU.add)
        nc.sync.dma_start(out=qb[it, b], in_=ci_all[:1, :])
        # 4 corner weights into one [1, 4T] row, one DMA to DRAM
        cw_all = ppool.tile([1, 4 * T], BF16, tag=f"cw{slot}",
                            name="cw_all")
        for k, (a_, b_) in enumerate(
            ((om["x"], om["y"]), (fr["x"], om["y"]),
             (om["x"], fr["y"]), (fr["x"], fr["y"]))
        ):
            nc.vector.tensor_tensor(cw_all[:1, k * T : (k + 1) * T],
                                    a_[:1, :], b_[:1, :], ALU.mult)
        yield

        # ---- rasterize: splat w [T=2x128 partitions, J] ----
        w_t = [
            wsplat.tile([128, J], BF16, tag=f"wt{slot}", name=f"w{h}")
            for h in range(2)
        ]
        for h in range(2):
            for c in range(8):
                pd = pbig.tile([128, 512], F32, tag="pb")
                nc.tensor.matmul(
                    pd[:, :],
                    lhsT_r[:, bt + h * 128 : bt + h * 128 + 128],
                    rhs_r[:, c * 512 : (c + 1) * 512],
                )
                nc.scalar.activation(
                    w_t[h][:, c * 512 : (c + 1) * 512], pd[:, :], AF.Exp,
                    scale=A_EXP,
                )
        yield

        # ---- wsum/vx/vy sums + heat (chunked) ----
        snb_all = sums32.tile([3, J], BF16, tag="snb", bufs=1,
                                name="snb_all")
        for c in range(8):
            pss = psums.tile([4, 512], F32, tag="pss")
            nc.tensor.matmul(
                pss[:3, :], lsums[0][:, :],
                w_t[0][:, c * 512 : (c + 1) * 512], start=True, stop=False,
            )
            nc.tensor.matmul(
                pss[:3, :], lsums[1][:, :],
                w_t[1][:, c * 512 : (c + 1) * 512], start=False, stop=True,
            )
            nc.scalar.copy(snb_all[:, c * 512 : (c + 1) * 512], pss[:3, :])
        wsum32 = sums32.tile([32, 128], BF16, tag=f"wsum32{slot}", bufs=1)
        vx32 = sums32.tile([32, 128], BF16, tag=f"vx32{slot}", bufs=1)
        vy32 = sums32.tile([32, 128], BF16, tag=f"vy32{slot}", bufs=1)
        for row, dst in ((0, vx32), (1, vy32), (2, wsum32)):
            nc.sync.dma_start(out=dst[:, :], in_=snb_all[row : row + 1, :])
        nc.vector.tensor_scalar(wsum32[:, :], wsum32[:, :], 1e-6, None,
                                ALU.max)
        rw32 = sums32.tile([32, 128], F32, tag=f"rw32{slot}", bufs=1)
        nc.vector.reciprocal(rw32[:, :], wsum32[:, :])
        vxn = sums32.tile([32, 128], FP8, tag=f"vxn{slot}", bufs=1)
        nc.vector.tensor_tensor(vxn[:, :], vx32[:, :], rw32[:, :], ALU.mult)
        vyn = sums32.tile([32, 128], FP8, tag=f"vyn{slot}", bufs=1)
        nc.vector.tensor_tensor(vyn[:, :], vy32[:, :], rw32[:, :], ALU.mult)
        nc.sync.dma_start(out=inp_b[b][65:66, :], in_=vxn[:, :])
        nc.sync.dma_start(out=inp_b[b][66:67, :], in_=vyn[:, :])
        # heat = max over t (chunked; reuses splat tiles — WAR deps order
        # each chunk after the sums matmuls read it)
        for c in range(8):
            sl = slice(c * 512, (c + 1) * 512)
            nc.vector.tensor_tensor(w_t[0][:, sl], w_t[0][:, sl],
                                    w_t[1][:, sl], ALU.max)
            nc.gpsimd.partition_all_reduce(
                w_t[1][:, sl], w_t[0][:, sl], channels=128,
                reduce_op=bass_isa.ReduceOp.max,
            )
            nc.vector.tensor_copy(inp_b[b][64:65, sl], w_t[1][0:1, sl])
        yield

        # ---- in_proj -> S_pad ----
        S_pad = spool.tile([128, NTILE], FP8, tag=f"spad{slot}",
                           name="S_pad")
        zero_pads(S_pad)
        for c in range(8):
            pip = pbig.tile([128, 512], F32, tag="pb")
            nc.tensor.matmul(
                pip[:, :], WtA[0:67, :], inp_b[b][:, c * 512 : (c + 1) * 512]
            )
            base = (8 * c + 1) * PW + PADOFF
            dst = S_pad[:, base : base + 8 * PW].rearrange(
                "p (r c) -> p r c", c=PW
            )[:, :, 1 : 1 + WS]
            nc.scalar.activation(
                dst, pip[:, :].rearrange("p (r c) -> p r c", c=WS),
                AF.Identity, bias=bias_S[:, b : b + 1], scale=1.0 / WSCALE,
            )
        yield

        # ---- residual blocks ----
        for blk in range(NB):
            h1 = hpool.tile([128, NTILE], FP8, tag=f"hh{slot}", name="h1")
            st6a = statp.tile([128, 9, 6], F32, tag=f"st6{slot}",
                              name="st6a")
            conv(h1, S_pad, conv_pair[blk][0],
                 gcols["c1b"][:, blk : blk + 1], st6a)
            yield
            sc1, bc1 = gn_coeffs(st6a, gcols["g1w"][:, blk : blk + 1],
                                 gcols["g1b"][:, blk : blk + 1], slot)
            for r0 in range(0, 64, 8):
                va = valid3(h1, r0, 8)
                nc.scalar.activation(va, va, AF.Silu, bias=bc1, scale=sc1)
            yield
            h2 = hpool.tile([128, NTILE], FP8, tag=f"hh{slot}", name="h2")
            st6b = statp.tile([128, 9, 6], F32, tag=f"st6{slot}",
                              name="st6b")
            conv(h2, h1, conv_pair[blk][1],
                 gcols["c2b"][:, blk : blk + 1], st6b)
            yield
            sc2, bc2 = gn_coeffs(st6b, gcols["g2w"][:, blk : blk + 1],
                                 gcols["g2b"][:, blk : blk + 1], slot)
            if blk == NB - 1:
                S32 = spool.tile([128, NTILE], F32, tag="s32",
                                 bufs=1, name="S32")
                zero_pads(S32)
            for r0 in range(0, 64, 8):
                vh2 = valid3(h2, r0, 8)
                vS = valid3(S_pad, r0, 8)
                nc.vector.tensor_scalar(vh2, vh2, sc2[:, :], None, ALU.mult)
                nc.vector.tensor_tensor(vS, vS, vh2, ALU.add)
                if blk < NB - 1:
                    nc.scalar.activation(vS, vS, AF.Silu, bias=bc2)
                else:
                    nc.scalar.activation(valid3(S32, r0, 8), vS, AF.Silu,
                                         bias=bc2)
            yield

        # ---- grid sample + out_proj + MLP ----
        # [4,T] u16 block -> [16, (k,hi)] i16 wraps, then doubling
        # replication to 128 partitions (7 DMAs vs 16 per-k)
        idxr = gpool.tile([128, 64], I16, tag=f"idx{slot}", name="idxr",
                          bufs=1)
        qs = qb[it, b]
        for k in range(4):
            nc.sync.dma_start(
                out=idxr[0:16, k * 16 : (k + 1) * 16],
                in_=bass.AP(tensor=qs.tensor, offset=qs.offset + k * T,
                            ap=[[1, 16], [16, 16]]).bitcast(I16),
            )
        nc.sync.dma_start(out=idxr[16:32, :], in_=idxr[0:16, :])
        nc.sync.dma_start(out=idxr[32:64, :], in_=idxr[0:32, :])
        nc.sync.dma_start(out=idxr[64:128, :], in_=idxr[0:64, :])
        # corner weights broadcast to 128 partitions on gpsimd (no DRAM
        # bounce)
        cwb = gpool.tile([128, 4 * T], BF16, tag=f"cwb{slot}", name="cwb",
                         bufs=1)
        nc.gpsimd.partition_broadcast(cwb[:, :], cw_all[:1, :], channels=128)
        gs = []
        for k in range(4):
            g = gpool.tile([128, T], F32, tag=f"g{slot}", name=f"g{k}",
                           bufs=2)
            nc.gpsimd.ap_gather(
                g[:, :], S32[:, :], idxr[:, k * 16 : (k + 1) * 16],
                channels=128, num_elems=NTILE, d=1, num_idxs=T,
            )
            gw = gpool.tile([128, T], BF16, tag=f"gw{k}{slot}",
                            name=f"gw{k}", bufs=1)
            nc.vector.tensor_tensor(gw[:, :], g[:, :],
                                    cwb[:, k * T : (k + 1) * T], ALU.mult)
            gs.append(gw)
        rp = gpool.tile([128, T], BF16, tag=f"rp{slot}", bufs=1)
        nc.vector.tensor_tensor(rp[:, :], gs[0][:, :], gs[1][:, :], ALU.add)
        rp2 = gpool.tile([128, T], BF16, tag=f"rp2{slot}", bufs=1)
        nc.vector.tensor_tensor(rp2[:, :], gs[2][:, :], gs[3][:, :],
                                ALU.add)
        nc.vector.tensor_tensor(rp[:, :], rp[:, :], rp2[:, :], ALU.add)
        yield

        pr = psmall.tile([128, 256], F32, tag="ps")
        nc.tensor.matmul(pr[:, :T], owT[:, :], rp[:, :])
        read_sb = gpool.tile([128, T], BF16, tag=f"read{slot}", bufs=1)
        nc.scalar.copy(read_sb[:, :], pr[:, :T])
        ph1 = psmall.tile([128, 256], F32, tag="ps")
        nc.tensor.matmul(ph1[:, :T], w1a[:, :], read_sb[:, :])
        h1s = gpool.tile([128, T], BF16, tag=f"h1s{slot}", bufs=1)
        nc.scalar.activation(h1s[:, :], ph1[:, :T], AF.Silu,
                             bias=tb1[:, b : b + 1])
        ph2 = psmall.tile([128, 256], F32, tag="ps")
        nc.tensor.matmul(ph2[:, :T], w2b[:, :], h1s[:, :])
        h2s = gpool.tile([128, T], BF16, tag=f"h2s{slot}", bufs=1)
        nc.scalar.activation(h2s[:, :], ph2[:, :T], AF.Silu,
                             bias=b2_c[:, :])
        for cd, xrow, prow in (("x", x_x, 0), ("y", x_y, 1)):
            pdx = psmall.tile([128, 256], F32, tag="ps")
            nc.tensor.matmul(pdx[:1, :T], w3b[:, prow : prow + 1],
                             h2s[:, :])
            ux = ppool.tile([1, T], F32, tag=f"ux{cd}", name=f"ux{cd}")
            nc.vector.tensor_scalar(
                ux[:1, :], pdx[0:1, :T],
                b3c2[cd][:, :], UPD, ALU.add, ALU.mult,
            )
            nc.vector.tensor_tensor(
                xrow[:1, bt : bt + T], xrow[:1, bt : bt + T], ux[:1, :],
                ALU.add,
            )
            nc.vector.tensor_scalar(
                xrow[:1, bt : bt + T], xrow[:1, bt : bt + T], -1.0, 1.0,
                ALU.max, ALU.min,
            )

    # ============ main refine loop: staggered 2-deep pipeline ============
    # Each step is a generator with 18 phase segments. Consecutive steps
    # start OFFSET segments apart, so one step's conv phase (PE-heavy)
    # overlaps the next step's rasterize phase (DVE/DMA-heavy). Two steps
    # are in flight at any time; slots alternate so their tiles don't
    # collide.
    steps = [(it, b) for it in range(R) for b in range(BL)]
    OFFSET = 9
    gens = [step(it, b, i % 2) for i, (it, b) in enumerate(steps)]
    done = [False] * len(gens)
    tick = 0
    while not all(done):
        for i, g in enumerate(gens):
            if not done[i] and tick >= i * OFFSET:
                try:
                    next(g)
                except StopIteration:
                    done[i] = True
        tick += 1

    # ---------------- output: x rows -> out [TB, 2] ----------------
    xpair = gpool.tile([2, TB], F32, tag="xpair")
    nc.sync.dma_start(out=xpair[0:1, :], in_=x_x[:1, :])
    nc.sync.dma_start(out=xpair[1:2, :], in_=x_y[:1, :])
    for c in range(8):
        pt = psmall.tile([128, 256], F32, tag="ps")
        nc.tensor.matmul(
            pt[:128, :2], xpair[:, c * 128 : (c + 1) * 128], id32[:2, :2],
            is_transpose=True,
        )
        ot = gpool.tile([128, 2], F32, tag="ot")
        nc.vector.tensor_copy(ot[:, :], pt[:128, :2])
        nc.sync.dma_start(out=io["out"][c * 128 : (c + 1) * 128, :],
                          in_=ot[:, :])


def build_nc():
    nc = bacc.Bacc("TRN2", target_bir_lowering=False, debug=False)
    io = {}
    io["M"] = nc.dram_tensor("M", [BL, CM, HIN * WIN], F32,
                             kind="ExternalInput").ap()
    io["x0"] = nc.dram_tensor("x0", [TB, 2], F32, kind="ExternalInput").ap()
    io["tE"] = nc.dram_tensor("tE", [BL, CT], F32, kind="ExternalInput").ap()
    io["ipw"] = nc.dram_tensor("ipw", [CS, CIN], F32, kind="ExternalInput").ap()
    io["ipb"] = nc.dram_tensor("ipb", [CS], F32, kind="ExternalInput").ap()
    for nm in ("c1w", "c2w"):
        io[nm] = nc.dram_tensor(nm, [NB, CS, CS * 9], F32,
                                kind="ExternalInput").ap()
    for nm in ("c1b", "g1w", "g1b", "c2b", "g2w", "g2b"):
        io[nm] = nc.dram_tensor(nm, [NB, CS], F32, kind="ExternalInput").ap()
    io["ow"] = nc.dram_tensor("ow", [CS, CS], F32, kind="ExternalInput").ap()
    io["ob"] = nc.dram_tensor("ob", [CS], F32, kind="ExternalInput").ap()
    io["w1"] = nc.dram_tensor("w1", [CS + CT, HID], F32,
                              kind="ExternalInput").ap()
    io["b1"] = nc.dram_tensor("b1", [HID], F32, kind="ExternalInput").ap()
    io["w2"] = nc.dram_tensor("w2", [HID, HID], F32, kind="ExternalInput").ap()
    io["b2"] = nc.dram_tensor("b2", [HID], F32, kind="ExternalInput").ap()
    io["w3"] = nc.dram_tensor("w3", [HID, 2], F32, kind="ExternalInput").ap()
    io["b3"] = nc.dram_tensor("b3", [2], F32, kind="ExternalInput").ap()
    io["out"] = nc.dram_tensor("out", [TB, 2], F32, kind="ExternalOutput").ap()
    io["qb"] = nc.dram_tensor("qb", [R, BL, 4 * T], U16).ap()

    with tile.TileContext(nc) as tc:
        with ExitStack() as ctx:
            emit(ctx, tc, io)
    nc.compile()
    return nc


def make_in_maps(inputs: dict) -> list[dict]:
    f = lambda x, d=np.float32: np.ascontiguousarray(np.asarray(x, d))
    weights = {
        "ipw": f(inputs["in_proj_w"]), "ipb": f(inputs["in_proj_b"]),
        "c1w": f(inputs["rb_c1w"]).reshape(NB, CS, CS * 9),
        "c1b": f(inputs["rb_c1b"]), "g1w": f(inputs["rb_g1w"]),
        "g1b": f(inputs["rb_g1b"]),
        "c2w": f(inputs["rb_c2w"]).reshape(NB, CS, CS * 9),
        "c2b": f(inputs["rb_c2b"]), "g2w": f(inputs["rb_g2w"]),
        "g2b": f(inputs["rb_g2b"]),
        "ow": f(inputs["out_w"]), "ob": f(inputs["out_b"]),
        "w1": f(inputs["mlp_w1"]), "b1": f(inputs["mlp_b1"]),
        "w2": f(inputs["mlp_w2"]), "b2": f(inputs["mlp_b2"]),
        "w3": f(inputs["mlp_w3"]), "b3": f(inputs["mlp_b3"]),
    }
    M = f(inputs["M"]).reshape(B_FULL, CM, HIN * WIN)
    x0 = f(inputs["x0_hat_norm"])
    tE = f(inputs["t_embed"])
    maps = []
    for c in range(NCORES):
        sl = slice(c * BL, (c + 1) * BL)
        m = dict(weights)
        m["M"] = np.ascontiguousarray(M[sl])
        m["x0"] = np.ascontiguousarray(x0[sl].reshape(TB, 2))
        m["tE"] = np.ascontiguousarray(tE[sl])
        maps.append(m)
    return maps


_NC_CACHE = {}


def kernel(**inputs) -> np.ndarray:
    if "nc" not in _NC_CACHE:
        _NC_CACHE["nc"] = build_nc()
    nc = _NC_CACHE["nc"]
    in_maps = make_in_maps(inputs)
    res = run_bass_kernel_spmd(nc, in_maps, core_ids=list(range(NCORES)))
    outs = [res.results[c]["out"].reshape(BL, T, 2) for c in range(NCORES)]
    return np.concatenate(outs, axis=0).astype(np.float32)


if __name__ == "__main__":
    nc = build_nc()
    n_inst = sum(len(getattr(f, "instructions", [])) for f in nc.m.functions)
    print(f"built ok, {n_inst} instructions")


# revision 3
# speedup vs baseline: 1.5826x; 1.5826x over previous
"""Trainium2 Bass kernel for BiDirectionalFusionBlock.

Data-parallel over batch: B=32 -> 8 cores x 4 local batch.
Per core, per refine iteration (R=3), per local batch element:
  1. rasterize: d2 via split-bf16 PE matmul -> ACT exp -> splat w [T,J];
     (wsum, vx, vy) via PE matmul over t; heat via DVE max +
     gpsimd partition_all_reduce.
  2. in_proj as 1x1-conv matmul (K=67: Ms 64ch + P 3ch; t_embed folded
     into a per-(b,out-ch) bias; 2x2 avg-pool of M folded into weights).
  3. 3 residual blocks: conv3x3 = 5 fp8e4m3 DoubleRow pair-matmuls (2
     taps per PE pass) on a 66-pitch zero-padded layout; weights
     pre-scaled by 32 into fp8 normal range (GroupNorm downstream is
     scale-invariant); GroupNorm via bn_stats/bn_aggr +
     indicator-matmul group reduce; SiLU fused into ACT affine pass.
  4. grid_sample: gpsimd ap_gather of 4 bilinear corners (zero padding
     free from the padded layout), out_proj + MLP on the 256 gathered
     points; x <- clip(x + 0.2*dx).

Two batch elements are software-pipelined (emission interleaved via
generators) so one batch's PE matmul stream fills the other's
GroupNorm/SiLU dependency stalls. Cross-engine row moves ([1,T] ->
rows of [10,TB]) go through one-hot PE matmuls instead of per-row
DMAs; the grid-sample index/weight DRAM bounces are single batched
DMAs; long SiLU/residual passes are chunked so consumers can start
early.
"""

from contextlib import ExitStack

import numpy as np

import concourse.bass as bass
import concourse.bacc as bacc
import concourse.tile as tile
from concourse import bass_isa, library_config, mybir
from concourse.bass_utils import run_bass_kernel_spmd

F32 = mybir.dt.float32
BF16 = mybir.dt.bfloat16
FP8 = mybir.dt.float8e4
I32 = mybir.dt.int32
U16 = mybir.dt.uint16
I16 = mybir.dt.int16
AF = mybir.ActivationFunctionType
ALU = mybir.AluOpType
PM_DR = mybir.MatmulPerfMode.DoubleRow

# conv weights are scaled up by WSCALE before the fp8e4m3 cast (raw std
# 0.02 sits in the subnormal range); GroupNorm right after each conv is
# scale-invariant so nothing needs un-scaling (conv bias is scaled too).
# 16 (not 32): raw conv outputs must stay below fp8e4m3 inf (|x|>255).
WSCALE = 16.0

# problem constants (hardcoded; kernel must be self-contained)
NCORES = 8
B_FULL = 32
BL = B_FULL // NCORES      # local batch = 4
T = 256
TB = BL * T                # 1024
CM, CP, CT, CS, HID = 64, 3, 128, 128, 128
CIN = CM + CP + CT         # 195
HS = WS = 64
J = HS * WS                # 4096
HIN = WIN = 128
R = 3
NB = 3
GROUPS = 8
GSZ = CS // GROUPS         # 16
EPS_GN = 1e-5
SIGMA = 1.2
UPD = 0.2
A_EXP = float(np.float32(-0.5) / np.float32(SIGMA * SIGMA + 1e-8))

# padded conv layout: 66-pitch rows + pad ring, stored at +1 (PADOFF) so
# corner taps of the first/last chunk stay in-bounds. p=(y+1)*66+(x+1)+1
PW = WS + 2                # 66
NPAD = PW * (HS + 2)       # 4356
PADOFF = 1
NTILE = NPAD + 2           # 4358
KROWS = 10                 # split-bf16 d2 matmul contraction rows
TAPOFF = [-PW - 1, -PW, -PW + 1, -1, 0, 1, PW - 1, PW, PW + 1]


def emit(ctx: ExitStack, tc: tile.TileContext, io: dict):
    nc = tc.nc

    singles = ctx.enter_context(tc.tile_pool(name="singles", bufs=1))
    psmall = ctx.enter_context(tc.tile_pool(name="psmall", bufs=2, space="PSUM"))

    def valid3(t, row0, nrows):
        """[128, nrows, 64] view of valid cells, padded rows row0..row0+nrows"""
        base = (row0 + 1) * PW + PADOFF
        sl = t[:, base : base + nrows * PW]
        return sl.rearrange("p (r c) -> p r c", c=PW)[:, :, 1 : 1 + WS]

    def zero_pads(t):
        nc.vector.memset(t[:, 0 : PADOFF + PW + 1], 0.0)
        nc.vector.memset(t[:, PADOFF + NPAD - PW : NTILE], 0.0)
        ring = t[:, PADOFF + PW : PADOFF + PW + 64 * PW].rearrange(
            "p (r c) -> p r c", c=PW
        )
        nc.vector.memset(ring[:, :, 0:1], 0.0)
        nc.vector.memset(ring[:, :, 65:66], 0.0)

    # =========== setup (stage/mload pools freed before main loop) ========
    # conv weights as fp8 DoubleRow pairs: pair p holds taps (2p, 2p+1),
    # pair 4 holds tap 8 + a zero plane. Tile [128, 256] = [i, (two o)].
    conv_pair = [[[None] * 5 for _ in range(2)] for _ in range(NB)]
    with ExitStack() as sctx:
        stage = sctx.enter_context(tc.tile_pool(name="stage", bufs=2))
        mload = sctx.enter_context(tc.tile_pool(name="mload", bufs=2))

        # identity matrices
        iden_i = stage.tile([128, 128], I32, tag="ideni")
        nc.gpsimd.iota(iden_i[:, :], pattern=[[1, 128]], base=0,
                       channel_multiplier=-1)
        iden_f = stage.tile([128, 128], F32, tag="idenf")
        nc.vector.tensor_copy(iden_f[:, :], iden_i[:, :])
        id32 = singles.tile([128, 128], F32)
        nc.vector.tensor_scalar(id32[:, :], iden_f[:, :], 0.0, None, ALU.is_equal)
        idbf = singles.tile([128, 128], BF16)
        nc.vector.tensor_copy(idbf[:, :], id32[:, :])

        # one-hot row-selector for PE row placement: window r = cols
        # [9r, 9r+8) holds e_r (a 1 at col 9r+r)
        sel = singles.tile([1, 128], BF16)
        nc.vector.memset(sel[:, :], 0.0)
        for r in range(8):
            nc.vector.memset(sel[:1, 9 * r + r : 9 * r + r + 1], 1.0)

        # conv weights: [o,(i 3 3)] -> fp8 pair lhsT tiles [i, (two, o)],
        # scaled by WSCALE (GN downstream is scale-invariant)
        for blk in range(NB):
            for cv, nm in enumerate(("c1w", "c2w")):
                st = stage.tile([128, 1152], F32, tag="wstage")
                nc.sync.dma_start(out=st[:, :], in_=io[nm][blk])
                stv = st[:, :].rearrange("p (i n) -> p i n", n=9)
                for p in range(5):
                    lt = singles.tile([128, 256], FP8, tag=f"cp{blk}{cv}{p}")
                    for h in range(2):
                        k = 2 * p + h
                        if k < 9:
                            pt = psmall.tile([128, 256], F32, tag="ps")
                            nc.tensor.matmul(
                                pt[:, :128], stv[:, :, k], id32[:, :],
                                is_transpose=True,
                            )
                            nc.scalar.activation(
                                lt[:, h * 128 : (h + 1) * 128], pt[:, :128],
                                AF.Identity, scale=WSCALE,
                            )
                        else:
                            nc.vector.memset(
                                lt[:, h * 128 : (h + 1) * 128], 0.0
                            )
                    conv_pair[blk][cv][p] = lt

        # in_proj W^T
        ipst = stage.tile([128, 195], F32, tag="ipst")
        nc.sync.dma_start(out=ipst[:, :], in_=io["ipw"][:, :])
        WtA = singles.tile([128, 128], FP8)      # rows i=0..127
        WtA32 = stage.tile([128, 128], F32, tag="wta32")
        WtB32 = stage.tile([67, 128], F32, tag="wtb32")
        ptA = psmall.tile([128, 256], F32, tag="ps")
        nc.tensor.matmul(ptA[:, :128], ipst[:, 0:128], id32[:, :],
                         is_transpose=True)
        nc.vector.tensor_copy(WtA32[:, :], ptA[:, :128])
        ptB = psmall.tile([128, 256], F32, tag="ps")
        nc.tensor.matmul(ptB[:67, :128], ipst[:, 128:195], id32[:, :],
                         is_transpose=True)
        nc.vector.tensor_copy(WtB32[:, :], ptB[:67, :128])
        # fp8 copy; fold 2x2 avg-pool 0.25 into Ms rows and WSCALE into
        # everything (un-scaled by the in_proj ACT's 1/WSCALE)
        nc.vector.tensor_scalar(WtA[0:64, :], WtA32[0:64, :], 0.25 * WSCALE,
                                None, ALU.mult)
        nc.vector.tensor_scalar(WtA[64:128, :], WtA32[64:128, :], WSCALE,
                                None, ALU.mult)

        # t_embed^T
        test_ = stage.tile([4, 128], F32, tag="test")
        nc.sync.dma_start(out=test_[:, :], in_=io["tE"][:, :])
        tEt = singles.tile([128, 4], F32)
        ptT = psmall.tile([128, 256], F32, tag="ps")
        nc.tensor.matmul(ptT[:, :4], test_[:, :], id32[:4, :4],
                         is_transpose=True)
        nc.vector.tensor_copy(tEt[:, :], ptT[:, :4])

        def col_from_vec(dram_ap, n, nm):
            row = stage.tile([1, 128], F32, tag="rowst")
            nc.sync.dma_start(out=row[:1, :n], in_=dram_ap.unsqueeze(0))
            col = singles.tile([n, 1], F32, tag=f"col_{nm}")
            pt = psmall.tile([128, 256], F32, tag="ps")
            nc.tensor.matmul(pt[:n, :1], row[:1, :n], id32[:1, :1],
                             is_transpose=True)
            nc.vector.tensor_copy(col[:, :], pt[:n, :1])
            return col

        ipb_c = col_from_vec(io["ipb"], 128, "ipb")
        ob_c = col_from_vec(io["ob"], 128, "ob")
        b1_c = col_from_vec(io["b1"], 128, "b1")
        b2_c = col_from_vec(io["b2"], 128, "b2")
        # b3 split per coordinate
        row3 = stage.tile([1, 128], F32, tag="rowst")
        nc.sync.dma_start(out=row3[:1, :2], in_=io["b3"].unsqueeze(0))
        b3x = singles.tile([1, 1], F32)
        nc.vector.tensor_copy(b3x[:, :], row3[0:1, 0:1])
        b3y = singles.tile([1, 1], F32)
        nc.vector.tensor_copy(b3y[:, :], row3[0:1, 1:2])
        b3c2 = {"x": b3x[:, :], "y": b3y[:, :]}

        # bias_S[o,b] = ipb + W^T[67:195]·tE_b   (t_map contribution)
        Wt_t = stage.tile([128, 128], F32, tag="wtt")
        ptW = psmall.tile([128, 256], F32, tag="ps")
        nc.tensor.matmul(ptW[:, :128], ipst[:, 67:195], id32[:, :],
                         is_transpose=True)
        nc.vector.tensor_copy(Wt_t[:, :], ptW[:, :128])
        pbs = psmall.tile([128, 256], F32, tag="ps")
        nc.tensor.matmul(pbs[:, :4], Wt_t[:, :], tEt[:, :])
        bias_S = singles.tile([128, 4], F32)
        nc.vector.tensor_scalar(bias_S[:, :], pbs[:, :4], ipb_c[:, :], None,
                                ALU.add)

        # mlp weights (memory layout is already lhsT)
        w1a32 = stage.tile([128, 128], F32, tag="w1a32")
        nc.sync.dma_start(out=w1a32[:, :], in_=io["w1"][0:128, :])
        w1b32 = stage.tile([128, 128], F32, tag="w1b32")
        nc.sync.dma_start(out=w1b32[:, :], in_=io["w1"][128:256, :])
        w1a = singles.tile([128, 128], BF16)
        nc.vector.tensor_copy(w1a[:, :], w1a32[:, :])
        w2st = stage.tile([128, 128], F32, tag="w2st")
        nc.sync.dma_start(out=w2st[:, :], in_=io["w2"][:, :])
        w2b = singles.tile([128, 128], BF16)
        nc.vector.tensor_copy(w2b[:, :], w2st[:, :])
        w3st = stage.tile([128, 2], F32, tag="w3st")
        nc.sync.dma_start(out=w3st[:, :], in_=io["w3"][:, :])
        w3b = singles.tile([128, 2], BF16)
        nc.vector.tensor_copy(w3b[:, :], w3st[:, :])
        owst = stage.tile([128, 128], F32, tag="owst")
        nc.sync.dma_start(out=owst[:, :], in_=io["ow"][:, :])
        owT = singles.tile([128, 128], BF16)
        pow_ = psmall.tile([128, 256], F32, tag="ps")
        nc.tensor.matmul(pow_[:, :128], owst[:, :], id32[:, :],
                         is_transpose=True)
        nc.scalar.copy(owT[:, :], pow_[:, :128])

        # tb1[h,b] = b1 + W1[128:256]^T tE_b + W1[0:128]^T out_b
        ob4 = singles.tile([128, 4], F32)
        for c in range(4):
            nc.vector.tensor_copy(ob4[:, c : c + 1], ob_c[:, :])
        ptb = psmall.tile([128, 256], F32, tag="ps")
        nc.tensor.matmul(ptb[:, :4], w1b32[:, :], tEt[:, :], start=True,
                         stop=False)
        nc.tensor.matmul(ptb[:, :4], w1a32[:, :], ob4[:, :], start=False,
                         stop=True)
        tb1 = singles.tile([128, 4], F32)
        nc.vector.tensor_scalar(tb1[:, :], ptb[:, :4], b1_c[:, :], None,
                                ALU.add)

        # gamma/beta/conv-bias columns: [3,128] dram -> [128,3] f32
        gcols = {}
        for nm in ("c1b", "g1w", "g1b", "c2b", "g2w", "g2b"):
            st = stage.tile([3, 128], F32, tag="gst")
            nc.sync.dma_start(out=st[:, :], in_=io[nm][:, :])
            col = singles.tile([128, 3], F32, tag=f"gc{nm}")
            pt = psmall.tile([128, 256], F32, tag="ps")
            nc.tensor.matmul(pt[:, :3], st[:, :], id32[:3, :3],
                             is_transpose=True)
            if nm in ("c1b", "c2b"):
                # conv bias rides the WSCALE'd conv output into GN
                nc.vector.tensor_scalar(col[:, :], pt[:, :3], WSCALE, None,
                                        ALU.mult)
            else:
                nc.vector.tensor_copy(col[:, :], pt[:, :3])
            gcols[nm] = col

        eps8 = singles.tile([8, 1], F32)
        nc.vector.memset(eps8[:, :], EPS_GN)
        # GN group-reduce indicator (iota: p in [16g, 16g+16)), with the
        # padded-column count scale folded in
        ind16 = singles.tile([128, 8], F32)
        ii1 = stage.tile([128, 8], I32, tag="ii1")
        nc.gpsimd.iota(ii1[:, :], pattern=[[-16, 8]], base=0,
                       channel_multiplier=1)
        if1 = stage.tile([128, 8], F32, tag="if1")
        nc.vector.tensor_scalar(if1[:, :], ii1[:, :], 0, None, ALU.is_ge)
        if2 = stage.tile([128, 8], F32, tag="if2")
        nc.vector.tensor_scalar(if2[:, :], ii1[:, :], 16,
                                (NTILE / 4096.0) / GSZ, ALU.is_lt, ALU.mult)
        nc.vector.tensor_tensor(ind16[:, :], if1[:, :], if2[:, :], ALU.mult)

        # rasterize static rhs rows [10, J] bf16, built in [32,128] layout:
        # k:      0     1     2     3     4     5     6    7    8    9
        # lhsT:   xh    yh    xh    yh    xl    yl    sqh  sql  1    1
        # rhs:    sxh   syh   sxl   syl   sxh   syh   1    1    ssh  ssl
        #   sx = -2*xs (split h+l), ss = xs^2+ys^2 (split h+l)
        rhs_r = singles.tile([KROWS, J], BF16)
        jx32 = stage.tile([32, 128], I32, tag="jx32")
        nc.gpsimd.iota(jx32[:, :], pattern=[[0, 2], [1, 64]], base=0,
                       channel_multiplier=0)
        jy32 = stage.tile([32, 128], I32, tag="jy32")
        nc.gpsimd.iota(jy32[:, :], pattern=[[1, 2], [0, 64]], base=0,
                       channel_multiplier=2)

        def split_rows(val32, rh, rl, scale):
            """val32 [32,128] f32 * scale -> bf16 h+l, DMA'd to rhs_r rows"""
            m2 = stage.tile([32, 128], F32, tag="spl_m2")
            nc.vector.tensor_scalar(m2[:, :], val32[:, :], scale, None, ALU.mult)
            hh = stage.tile([32, 128], BF16, tag="spl_h")
            nc.vector.tensor_copy(hh[:, :], m2[:, :])
            rr = stage.tile([32, 128], F32, tag="spl_r")
            nc.vector.tensor_tensor(rr[:, :], m2[:, :], hh[:, :], ALU.subtract)
            ll = stage.tile([32, 128], BF16, tag="spl_l")
            nc.vector.tensor_copy(ll[:, :], rr[:, :])
            for r, tl in ((rh, hh), (rl, ll)):
                nc.sync.dma_start(out=rhs_r[r : r + 1, :], in_=tl[:, :])

        jxf = stage.tile([32, 128], F32, tag="jxf")
        nc.vector.tensor_copy(jxf[:, :], jx32[:, :])
        jyf = stage.tile([32, 128], F32, tag="jyf")
        nc.vector.tensor_copy(jyf[:, :], jy32[:, :])
        split_rows(jxf, 0, 2, -2.0)
        split_rows(jyf, 1, 3, -2.0)
        nc.sync.dma_start(out=rhs_r[4:5, :], in_=rhs_r[0:1, :])
        nc.sync.dma_start(out=rhs_r[5:6, :], in_=rhs_r[1:2, :])
        ones_bf = singles.tile([32, 128], BF16)
        nc.vector.memset(ones_bf[:, :], 1.0)
        nc.sync.dma_start(out=rhs_r[6:7, :], in_=ones_bf[:, :])
        nc.sync.dma_start(out=rhs_r[7:8, :], in_=ones_bf[:, :])
        ss32 = stage.tile([32, 128], F32, tag="ss32")
        nc.vector.tensor_tensor(ss32[:, :], jxf[:, :], jxf[:, :], ALU.mult)
        sy32 = stage.tile([32, 128], F32, tag="sy32")
        nc.vector.tensor_tensor(sy32[:, :], jyf[:, :], jyf[:, :], ALU.mult)
        nc.vector.tensor_tensor(ss32[:, :], ss32[:, :], sy32[:, :], ALU.add)
        split_rows(ss32, 8, 9, 1.0)

        # x0 [TB,2] -> x_x / x_y [1,TB] f32 (separate so all ops are base-0)
        x_x = singles.tile([1, TB], F32)
        x_y = singles.tile([1, TB], F32)
        for c in range(8):
            xst = stage.tile([128, 2], F32, tag="xst")
            nc.sync.dma_start(out=xst[:, :],
                              in_=io["x0"][c * 128 : (c + 1) * 128, :])
            for d, xrow in ((0, x_x), (1, x_y)):
                pt = psmall.tile([128, 256], F32, tag="ps")
                nc.tensor.matmul(pt[:1, :128], xst[:, d : d + 1], id32[:, :],
                                 is_transpose=True)
                nc.vector.tensor_copy(xrow[:1, c * 128 : (c + 1) * 128],
                                      pt[0:1, :128])

        # M load + 2x2 pool (sum; 0.25 folded in WtA)
        inp_b = [
            singles.tile([67, J], FP8, tag=f"inp{b}", name=f"inp{b}")
            for b in range(BL)
        ]
        for bp in range(BL // 2):  # b-pairs stacked on 128 partitions
            for ch in range(8):
                mt = mload.tile([128, 2048], F32, tag="mt")
                src = io["M"][2 * bp : 2 * bp + 2, :,
                              ch * 2048 : (ch + 1) * 2048]
                nc.sync.dma_start(out=mt[:, :],
                                  in_=src.rearrange("b c f -> (b c) f"))
                a1 = mload.tile([128, 1024], F32, tag="a1")
                mv = mt[:, :].rearrange("p (y q x) -> p y q x", q=2, x=64)
                nc.vector.tensor_tensor(
                    a1[:, :].rearrange("p (y x) -> p y x", x=64),
                    mv[:, :, 0, :], mv[:, :, 1, :], ALU.add,
                )
                av = a1[:, :].rearrange("p (y q x) -> p y q x", q=2, x=64)
                for h in range(2):
                    b = 2 * bp + h
                    dst = inp_b[b][0:64, ch * 512 : (ch + 1) * 512]
                    nc.vector.tensor_tensor(
                        dst.rearrange("p (y x) -> p y x", x=64),
                        av[h * 64 : h * 64 + 64, :, 0, :],
                        av[h * 64 : h * 64 + 64, :, 1, :], ALU.add,
                    )

    # persistent pools (entered after setup scratch is released)
    wsplat = ctx.enter_context(tc.tile_pool(name="wsplat", bufs=2))
    spool = ctx.enter_context(tc.tile_pool(name="spool", bufs=2))
    hpool = ctx.enter_context(tc.tile_pool(name="hpool", bufs=2))
    statp = ctx.enter_context(tc.tile_pool(name="statp", bufs=2))
    ppool = ctx.enter_context(tc.tile_pool(name="ppool", bufs=1))
    gpool = ctx.enter_context(tc.tile_pool(name="gpool", bufs=2))
    sums32 = ctx.enter_context(tc.tile_pool(name="sums32", bufs=2))
    pbig = ctx.enter_context(tc.tile_pool(name="pbig", bufs=3, space="PSUM"))
    psums = ctx.enter_context(tc.tile_pool(name="psums", bufs=2, space="PSUM"))

    # dram bounce for the grid-sample index wrap ([R, BL, 4*T])
    qb = io["qb"]

    # dynamic raster lhsT [10, TB]; rows 8,9 = ones
    lhsT_r = singles.tile([KROWS, TB], BF16)
    nc.sync.dma_start(out=lhsT_r[8:9, :], in_=ones_bf[:8, :])
    nc.sync.dma_start(out=lhsT_r[9:10, :], in_=ones_bf[:8, :])

    # ---------------- per-(it,b) step as a generator ----------------
    # GN: one PE matmul for the group sums, then a replicating DMA
    # ([8,2] -> [128,2]) instead of a second PE matmul, so the conv
    # stream behind it in the PE queue never waits on the DVE round-trip.
    def gn_coeffs(stat6, gamma, beta, slot):
        mv_ = statp.tile([128, 2], F32, tag=f"mv{slot}", name="mv_")
        nc.vector.bn_aggr(mv_[:, :], stat6[:, :, :])
        st2 = statp.tile([128, 2], F32, tag=f"st2{slot}", name="st2")
        nc.vector.tensor_copy(st2[:, 0:1], mv_[:, 0:1])
        nc.vector.tensor_tensor(st2[:, 1:2], mv_[:, 0:1], mv_[:, 0:1],
                                ALU.mult)
        nc.vector.tensor_tensor(st2[:, 1:2], st2[:, 1:2], mv_[:, 1:2],
                                ALU.add)
        pg = psmall.tile([128, 256], F32, tag="ps")
        nc.tensor.matmul(pg[:8, :2], ind16[:, :], st2[:, :])
        g8 = statp.tile([8, 2], F32, tag=f"g8{slot}", name="g8")
        nc.vector.tensor_copy(g8[:, :], pg[:8, :2])
        g2 = statp.tile([8, 2], F32, tag=f"g2{slot}", name="g2")
        gmsq = statp.tile([8, 1], F32, tag=f"gmsq{slot}", name="gmsq")
        nc.vector.tensor_copy(g2[:, 0:1], g8[:, 0:1])
        nc.vector.tensor_tensor(gmsq[:, :], g8[:, 0:1], g8[:, 0:1],
                                ALU.mult)
        gvar = statp.tile([8, 1], F32, tag=f"gvar{slot}", name="gvar")
        nc.vector.tensor_tensor(gvar[:, :], g8[:, 1:2], gmsq[:, :],
                                ALU.subtract)
        gstd = statp.tile([8, 1], F32, tag=f"gstd{slot}", name="gstd")
        nc.scalar.activation(gstd[:, :], gvar[:, :], AF.Sqrt,
                             bias=eps8[:, :])
        nc.vector.reciprocal(g2[:, 1:2], gstd[:, :])
        ex = statp.tile([128, 2], F32, tag=f"ex{slot}", name="ex")
        g2s = g2[0:8, 0:2]
        nc.sync.dma_start(
            out=ex[:, :],
            in_=bass.AP(tensor=g2s.tensor, offset=g2s.offset,
                        ap=[[g2s.ap[0][0], 8], [0, GSZ], [1, 2]]),
        )
        sc = statp.tile([128, 1], F32, tag=f"sc{slot}", name="sc")
        nc.vector.tensor_tensor(sc[:, :], ex[:, 1:2], gamma, ALU.mult)
        bc = statp.tile([128, 1], F32, tag=f"bc{slot}", name="bc")
        nc.vector.tensor_tensor(bc[:, :], ex[:, 0:1], sc[:, :],
                                ALU.mult)
        nc.vector.tensor_tensor(bc[:, :], beta, bc[:, :], ALU.subtract)
        return sc, bc

    def conv(dst_pad, src_pad, pairs, bias_col, stat6):
        """conv3x3 via 5 fp8 DoubleRow pair-matmuls per 7-row chunk"""
        zero_pads(dst_pad)
        pitch = src_pad[:, 0:1].ap[0][0]
        for c in range(10):
            rc = 7 if c < 9 else 1
            base = (7 * c + 1) * PW + PADOFF
            n = rc * PW
            pc = pbig.tile([128, 512], F32, tag="pb")
            for p in range(5):
                ka = 2 * p
                off_a = base + TAPOFF[ka]
                delta = (TAPOFF[ka + 1] - TAPOFF[ka]) if p < 4 else 0
                s = src_pad[:, off_a : off_a + 1]
                rhs2 = bass.AP(
                    tensor=s.tensor, offset=s.offset,
                    ap=[[pitch, 128], [delta, 2], [1, n]],
                )
                nc.tensor.matmul(
                    pc[:, :n],
                    pairs[p][:, :].rearrange("q (two m) -> q two m", two=2),
                    rhs2,
                    start=(p == 0), stop=(p == 4),
                    perf_mode=PM_DR,
                )
            vout = valid3(dst_pad, 7 * c, rc)
            vin = pc[:, :n].rearrange("p (r c) -> p r c", c=PW)[
                :, :, 1 : 1 + WS
            ]
            nc.scalar.activation(vout, vin, AF.Identity, bias=bias_col)
        # stats over full padded rows (pads are zero; count scale is
        # folded into ind16)
        for c in range(9):
            lo = c * 512
            hi = min(NTILE, lo + 512)
            nc.vector.bn_stats(stat6[:, c, :], dst_pad[:, lo:hi])

    def step(it, b, slot):
        bt = b * T
        xs_x = x_x[:1, bt : bt + T]
        xs_y = x_y[:1, bt : bt + T]
        # ---- point prep (per b, [1, T] base-0 tiles) ----
        px = ppool.tile([1, T], F32, tag="px", name="px")
        nc.vector.tensor_scalar(px[:1, :], xs_x, 31.5, 31.5, ALU.mult,
                                ALU.add)
        py = ppool.tile([1, T], F32, tag="py", name="py")
        nc.vector.tensor_scalar(py[:1, :], xs_y, 31.5, 31.5, ALU.mult,
                                ALU.add)
        vx = ppool.tile([1, T], F32, tag=f"vx{slot}", name="vx")
        nc.vector.tensor_tensor(vx[:1, 1:T], px[:1, 1:T], px[:1, 0 : T - 1],
                                ALU.subtract)
        nc.vector.memset(vx[:1, 0:1], 0.0)
        vy = ppool.tile([1, T], F32, tag=f"vy{slot}", name="vy")
        nc.vector.tensor_tensor(vy[:1, 1:T], py[:1, 1:T], py[:1, 0 : T - 1],
                                ALU.subtract)
        nc.vector.memset(vy[:1, 0:1], 0.0)
        # lsums[t, (h, 0:2)] = (vx,vy) transposed per half; col 2 = ones.
        # fp8 so the sums matmul can DoubleRow both halves in one pass.
        lsums = sums32.tile([128, 32], FP8, tag=f"ls{slot}", name="lsums")
        for h in range(2):
            for d, src in ((0, vx), (1, vy)):
                pt = psmall.tile([128, 256], F32, tag="ps")
                nc.tensor.matmul(pt[:128, :1],
                                 src[:1, h * 128 : h * 128 + 128],
                                 id32[:1, :1], is_transpose=True)
                nc.vector.tensor_copy(lsums[:, 16 * h + d : 16 * h + d + 1],
                                      pt[:128, 0:1])
            nc.vector.memset(lsums[:, 16 * h + 2 : 16 * h + 3], 1.0)
        # split-bf16 rows of lhsT_r, placed via one-hot PE matmuls
        xh = ppool.tile([1, T], BF16, tag=f"xh{slot}", name="xh")
        nc.vector.tensor_copy(xh[:1, :], px[:1, :])
        yh = ppool.tile([1, T], BF16, tag=f"yh{slot}", name="yh")
        nc.vector.tensor_copy(yh[:1, :], py[:1, :])
        t1 = ppool.tile([1, T], F32, tag="t1", name="t1")
        nc.vector.tensor_tensor(t1[:1, :], px[:1, :], xh[:1, :],
                                ALU.subtract)
        xl = ppool.tile([1, T], BF16, tag=f"xl{slot}", name="xl")
        nc.vector.tensor_copy(xl[:1, :], t1[:1, :])
        t2 = ppool.tile([1, T], F32, tag="t2", name="t2")
        nc.vector.tensor_tensor(t2[:1, :], py[:1, :], yh[:1, :],
                                ALU.subtract)
        yl = ppool.tile([1, T], BF16, tag=f"yl{slot}", name="yl")
        nc.vector.tensor_copy(yl[:1, :], t2[:1, :])
        s1 = ppool.tile([1, T], F32, tag="s1", name="s1")
        nc.vector.tensor_tensor(s1[:1, :], px[:1, :], px[:1, :], ALU.mult)
        s2 = ppool.tile([1, T], F32, tag="s2", name="s2")
        nc.vector.tensor_tensor(s2[:1, :], py[:1, :], py[:1, :], ALU.mult)
        nc.vector.tensor_tensor(s1[:1, :], s1[:1, :], s2[:1, :], ALU.add)
        sqh = ppool.tile([1, T], BF16, tag=f"sqh{slot}", name="sqh")
        nc.vector.tensor_copy(sqh[:1, :], s1[:1, :])
        nc.vector.tensor_tensor(s2[:1, :], s1[:1, :], sqh[:1, :],
                                ALU.subtract)
        sql = ppool.tile([1, T], BF16, tag=f"sql{slot}", name="sql")
        nc.vector.tensor_copy(sql[:1, :], s2[:1, :])
        ps8 = psums.tile([8, 512], F32, tag="ps8", bufs=1, name="ps8")
        rowsrc = (xh, yh, xh, yh, xl, yl, sqh, sql)
        for r in range(8):
            nc.tensor.matmul(
                ps8[:8, :T], sel[0:1, 9 * r : 9 * r + 8], rowsrc[r][:1, :],
                start=(r == 0), stop=(r == 7),
            )
        nc.scalar.copy(lhsT_r[0:8, bt : bt + T], ps8[:8, :T])
        # grid-sample coords (floor + frac), per coordinate
        fr = {}
        om = {}
        fl = {}
        for cd, xs in (("x", xs_x), ("y", xs_y)):
            g_ = ppool.tile([1, T], F32, tag=f"g{cd}", name=f"g{cd}")
            nc.vector.tensor_scalar(g_[:1, :], xs, 32.0, 31.5, ALU.mult,
                                    ALU.add)
            xi = ppool.tile([1, T], I32, tag="xi", name="xi")
            nc.vector.tensor_copy(xi[:1, :], g_[:1, :])
            f_ = ppool.tile([1, T], F32, tag=f"f{cd}", name=f"f{cd}")
            nc.vector.tensor_copy(f_[:1, :], xi[:1, :])
            gt_ = ppool.tile([1, T], F32, tag="gt", name="gt")
            nc.vector.tensor_tensor(gt_[:1, :], f_[:1, :], g_[:1, :],
                                    ALU.is_gt)
            nc.vector.tensor_tensor(f_[:1, :], f_[:1, :], gt_[:1, :],
                                    ALU.subtract)
            nc.vector.tensor_tensor(g_[:1, :], g_[:1, :], f_[:1, :],
                                    ALU.subtract)
            o_ = ppool.tile([1, T], F32, tag=f"om{cd}", name=f"om{cd}")
            nc.vector.tensor_scalar(o_[:1, :], g_[:1, :], -1.0, 1.0,
                                    ALU.mult, ALU.add)
            fr[cd], om[cd], fl[cd] = g_, o_, f_
        q1 = ppool.tile([1, T], F32, tag="q1", name="q1")
        nc.vector.tensor_scalar(q1[:1, :], fl["y"][:1, :], 66.0,
                                float(67 + PADOFF), ALU.mult, ALU.add)
        q2 = ppool.tile([1, T], F32, tag="q2", name="q2")
        nc.vector.tensor_tensor(q2[:1, :], q1[:1, :], fl["x"][:1, :],
                                ALU.add)
        # 4 corner indices into one [1, 4T] row, one DMA to DRAM
        ci_all = ppool.tile([1, 4 * T], U16, tag="ci", name="ci_all")
        nc.vector.tensor_copy(ci_all[:1, 0:T], q2[:1, :])
        for k, off in ((1, 1), (2, 66), (3, 67)):
            nc.vector.tensor_scalar(ci_all[:1, k * T : (k + 1) * T],
                                    ci_all[:1, 0:T], off, None, ALU.add)
        nc.sync.dma_start(out=qb[it, b], in_=ci_all[:1, :])
        # 4 corner weights into one [1, 4T] row, one DMA to DRAM
        cw_all = ppool.tile([1, 4 * T], BF16, tag=f"cw{slot}",
                            name="cw_all")
        for k, (a_, b_) in enumerate(
            ((om["x"], om["y"]), (fr["x"], om["y"]),
             (om["x"], fr["y"]), (fr["x"], fr["y"]))
        ):
            nc.vector.tensor_tensor(cw_all[:1, k * T : (k + 1) * T],
                                    a_[:1, :], b_[:1, :], ALU.mult)
        yield

        # ---- rasterize: splat w [T=2x128 partitions, J], fp8 in one
        # [128, 2J] tile so the sums matmul can DoubleRow-pair the halves
        w2 = wsplat.tile([128, 2 * J], FP8, tag=f"wt{slot}", name="w2")
        w_t = [w2[:, h * J : (h + 1) * J] for h in range(2)]
        for h in range(2):
            for c in range(8):
                pd = pbig.tile([128, 512], F32, tag="pb")
                nc.tensor.matmul(
                    pd[:, :],
                    lhsT_r[:, bt + h * 128 : bt + h * 128 + 128],
                    rhs_r[:, c * 512 : (c + 1) * 512],
                )
                nc.scalar.activation(
                    w_t[h][:, c * 512 : (c + 1) * 512], pd[:, :], AF.Exp,
                    scale=A_EXP,
                )
        yield

        # ---- wsum/vx/vy sums + heat (chunked) ----
        snb_all = sums32.tile([3, J], BF16, tag="snb", bufs=1,
                                name="snb_all")
        w2pitch = w2[:, 0:1].ap[0][0]
        for c in range(8):
            pss = psums.tile([4, 512], F32, tag="pss")
            s = w2[:, c * 512 : c * 512 + 1]
            rhs2 = bass.AP(tensor=s.tensor, offset=s.offset,
                           ap=[[w2pitch, 128], [J, 2], [1, 512]])
            l0 = lsums[:, 0:1]
            lhs2 = bass.AP(tensor=l0.tensor, offset=l0.offset,
                           ap=[[l0.ap[0][0], 128], [16, 2], [1, 3]])
            nc.tensor.matmul(
                pss[:3, :], lhs2, rhs2,
                start=True, stop=True, perf_mode=PM_DR,
            )
            nc.scalar.copy(snb_all[:, c * 512 : (c + 1) * 512], pss[:3, :])
        wsum32 = sums32.tile([32, 128], BF16, tag=f"wsum32{slot}", bufs=1)
        vx32 = sums32.tile([32, 128], BF16, tag=f"vx32{slot}", bufs=1)
        vy32 = sums32.tile([32, 128], BF16, tag=f"vy32{slot}", bufs=1)
        for row, dst in ((0, vx32), (1, vy32), (2, wsum32)):
            nc.sync.dma_start(out=dst[:, :], in_=snb_all[row : row + 1, :])
        nc.vector.tensor_scalar(wsum32[:, :], wsum32[:, :], 1e-6, None,
                                ALU.max)
        rw32 = sums32.tile([32, 128], F32, tag=f"rw32{slot}", bufs=1)
        nc.vector.reciprocal(rw32[:, :], wsum32[:, :])
        vxn = sums32.tile([32, 128], FP8, tag=f"vxn{slot}", bufs=1)
        nc.vector.tensor_tensor(vxn[:, :], vx32[:, :], rw32[:, :], ALU.mult)
        vyn = sums32.tile([32, 128], FP8, tag=f"vyn{slot}", bufs=1)
        nc.vector.tensor_tensor(vyn[:, :], vy32[:, :], rw32[:, :], ALU.mult)
        nc.sync.dma_start(out=inp_b[b][65:66, :], in_=vxn[:, :])
        nc.sync.dma_start(out=inp_b[b][66:67, :], in_=vyn[:, :])
        # heat = max over t (chunked; reuses splat tiles — WAR deps order
        # each chunk after the sums matmuls read it)
        for c in range(8):
            sl = slice(c * 512, (c + 1) * 512)
            nc.vector.tensor_tensor(w_t[0][:, sl], w_t[0][:, sl],
                                    w_t[1][:, sl], ALU.max)
            nc.gpsimd.partition_all_reduce(
                w_t[1][:, sl], w_t[0][:, sl], channels=128,
                reduce_op=bass_isa.ReduceOp.max,
            )
            nc.vector.tensor_copy(inp_b[b][64:65, sl], w_t[1][0:1, sl])
        yield

        # ---- in_proj -> S_pad ----
        S_pad = spool.tile([128, NTILE], FP8, tag=f"spad{slot}",
                           name="S_pad")
        zero_pads(S_pad)
        for c in range(8):
            pip = pbig.tile([128, 512], F32, tag="pb")
            nc.tensor.matmul(
                pip[:, :], WtA[0:67, :], inp_b[b][:, c * 512 : (c + 1) * 512]
            )
            base = (8 * c + 1) * PW + PADOFF
            dst = S_pad[:, base : base + 8 * PW].rearrange(
                "p (r c) -> p r c", c=PW
            )[:, :, 1 : 1 + WS]
            nc.scalar.activation(
                dst, pip[:, :].rearrange("p (r c) -> p r c", c=WS),
                AF.Identity, bias=bias_S[:, b : b + 1], scale=1.0 / WSCALE,
            )
        yield

        # ---- residual blocks ----
        for blk in range(NB):
            h1 = hpool.tile([128, NTILE], FP8, tag=f"hh{slot}", name="h1")
            st6a = statp.tile([128, 9, 6], F32, tag=f"st6{slot}",
                              name="st6a")
            conv(h1, S_pad, conv_pair[blk][0],
                 gcols["c1b"][:, blk : blk + 1], st6a)
            yield
            sc1, bc1 = gn_coeffs(st6a, gcols["g1w"][:, blk : blk + 1],
                                 gcols["g1b"][:, blk : blk + 1], slot)
            for r0 in range(0, 64, 8):
                va = valid3(h1, r0, 8)
                nc.scalar.activation(va, va, AF.Silu, bias=bc1, scale=sc1)
            yield
            h2 = hpool.tile([128, NTILE], FP8, tag=f"hh{slot}", name="h2")
            st6b = statp.tile([128, 9, 6], F32, tag=f"st6{slot}",
                              name="st6b")
            conv(h2, h1, conv_pair[blk][1],
                 gcols["c2b"][:, blk : blk + 1], st6b)
            yield
            sc2, bc2 = gn_coeffs(st6b, gcols["g2w"][:, blk : blk + 1],
                                 gcols["g2b"][:, blk : blk + 1], slot)
            if blk == NB - 1:
                S32 = spool.tile([128, NTILE], F32, tag="s32",
                                 bufs=1, name="S32")
                zero_pads(S32)
            for r0 in range(0, 64, 8):
                vh2 = valid3(h2, r0, 8)
                vS = valid3(S_pad, r0, 8)
                nc.vector.tensor_scalar(vh2, vh2, sc2[:, :], None, ALU.mult)
                nc.vector.tensor_tensor(vS, vS, vh2, ALU.add)
                if blk < NB - 1:
                    nc.scalar.activation(vS, vS, AF.Silu, bias=bc2)
                else:
                    nc.scalar.activation(valid3(S32, r0, 8), vS, AF.Silu,
                                         bias=bc2)
            yield

        # ---- grid sample + out_proj + MLP ----
        # [4,T] u16 block -> [16, (k,hi)] i16 wraps, then doubling
        # replication to 128 partitions (7 DMAs vs 16 per-k)
        idxr = gpool.tile([128, 64], I16, tag=f"idx{slot}", name="idxr",
                          bufs=1)
        qs = qb[it, b]
        for k in range(4):
            nc.sync.dma_start(
                out=idxr[0:16, k * 16 : (k + 1) * 16],
                in_=bass.AP(tensor=qs.tensor, offset=qs.offset + k * T,
                            ap=[[1, 16], [16, 16]]).bitcast(I16),
            )
        nc.sync.dma_start(out=idxr[16:32, :], in_=idxr[0:16, :])
        nc.sync.dma_start(out=idxr[32:64, :], in_=idxr[0:32, :])
        nc.sync.dma_start(out=idxr[64:128, :], in_=idxr[0:64, :])
        # corner weights broadcast to 128 partitions on gpsimd (no DRAM
        # bounce)
        cwb = gpool.tile([128, 4 * T], BF16, tag=f"cwb{slot}", name="cwb",
                         bufs=1)
        nc.gpsimd.partition_broadcast(cwb[:, :], cw_all[:1, :], channels=128)
        gs = []
        for k in range(4):
            g = gpool.tile([128, T], F32, tag=f"g{slot}", name=f"g{k}",
                           bufs=2)
            nc.gpsimd.ap_gather(
                g[:, :], S32[:, :], idxr[:, k * 16 : (k + 1) * 16],
                channels=128, num_elems=NTILE, d=1, num_idxs=T,
            )
            gw = gpool.tile([128, T], BF16, tag=f"gw{k}{slot}",
                            name=f"gw{k}", bufs=1)
            nc.vector.tensor_tensor(gw[:, :], g[:, :],
                                    cwb[:, k * T : (k + 1) * T], ALU.mult)
            gs.append(gw)
        rp = gpool.tile([128, T], BF16, tag=f"rp{slot}", bufs=1)
        nc.vector.tensor_tensor(rp[:, :], gs[0][:, :], gs[1][:, :], ALU.add)
        rp2 = gpool.tile([128, T], BF16, tag=f"rp2{slot}", bufs=1)
        nc.vector.tensor_tensor(rp2[:, :], gs[2][:, :], gs[3][:, :],
                                ALU.add)
        nc.vector.tensor_tensor(rp[:, :], rp[:, :], rp2[:, :], ALU.add)
        yield

        pr = psmall.tile([128, 256], F32, tag="ps")
        nc.tensor.matmul(pr[:, :T], owT[:, :], rp[:, :])
        read_sb = gpool.tile([128, T], BF16, tag=f"read{slot}", bufs=1)
        nc.scalar.copy(read_sb[:, :], pr[:, :T])
        ph1 = psmall.tile([128, 256], F32, tag="ps")
        nc.tensor.matmul(ph1[:, :T], w1a[:, :], read_sb[:, :])
        h1s = gpool.tile([128, T], BF16, tag=f"h1s{slot}", bufs=1)
        nc.scalar.activation(h1s[:, :], ph1[:, :T], AF.Silu,
                             bias=tb1[:, b : b + 1])
        ph2 = psmall.tile([128, 256], F32, tag="ps")
        nc.tensor.matmul(ph2[:, :T], w2b[:, :], h1s[:, :])
        h2s = gpool.tile([128, T], BF16, tag=f"h2s{slot}", bufs=1)
        nc.scalar.activation(h2s[:, :], ph2[:, :T], AF.Silu,
                             bias=b2_c[:, :])
        for cd, xrow, prow in (("x", x_x, 0), ("y", x_y, 1)):
            pdx = psmall.tile([128, 256], F32, tag="ps")
            nc.tensor.matmul(pdx[:1, :T], w3b[:, prow : prow + 1],
                             h2s[:, :])
            ux = ppool.tile([1, T], F32, tag=f"ux{cd}", name=f"ux{cd}")
            nc.vector.tensor_scalar(
                ux[:1, :], pdx[0:1, :T],
                b3c2[cd][:, :], UPD, ALU.add, ALU.mult,
            )
            nc.vector.tensor_tensor(
                xrow[:1, bt : bt + T], xrow[:1, bt : bt + T], ux[:1, :],
                ALU.add,
            )
            nc.vector.tensor_scalar(
                xrow[:1, bt : bt + T], xrow[:1, bt : bt + T], -1.0, 1.0,
                ALU.max, ALU.min,
            )

    # ============ main refine loop: staggered 2-deep pipeline ============
    # Each step is a generator with 18 phase segments. Consecutive steps
    # start OFFSET segments apart, so one step's conv phase (PE-heavy)
    # overlaps the next step's rasterize phase (DVE/DMA-heavy). Two steps
    # are in flight at any time; slots alternate so their tiles don't
    # collide.
    steps = [(it, b) for it in range(R) for b in range(BL)]
    OFFSET = 9
    gens = [step(it, b, i % 2) for i, (it, b) in enumerate(steps)]
    done = [False] * len(gens)
    tick = 0
    while not all(done):
        for i, g in enumerate(gens):
            if not done[i] and tick >= i * OFFSET:
                try:
                    next(g)
                except StopIteration:
                    done[i] = True
        tick += 1

    # ---------------- output: x rows -> out [TB, 2] ----------------
    xpair = gpool.tile([2, TB], F32, tag="xpair")
    nc.sync.dma_start(out=xpair[0:1, :], in_=x_x[:1, :])
    nc.sync.dma_start(out=xpair[1:2, :], in_=x_y[:1, :])
    for c in range(8):
        pt = psmall.tile([128, 256], F32, tag="ps")
        nc.tensor.matmul(
            pt[:128, :2], xpair[:, c * 128 : (c + 1) * 128], id32[:2, :2],
            is_transpose=True,
        )
        ot = gpool.tile([128, 2], F32, tag="ot")
        nc.vector.tensor_copy(ot[:, :], pt[:128, :2])
        nc.sync.dma_start(out=io["out"][c * 128 : (c + 1) * 128, :],
                          in_=ot[:, :])


def build_nc():
    nc = bacc.Bacc("TRN2", target_bir_lowering=False, debug=False)
    io = {}
    io["M"] = nc.dram_tensor("M", [BL, CM, HIN * WIN], F32,
                             kind="ExternalInput").ap()
    io["x0"] = nc.dram_tensor("x0", [TB, 2], F32, kind="ExternalInput").ap()
    io["tE"] = nc.dram_tensor("tE", [BL, CT], F32, kind="ExternalInput").ap()
    io["ipw"] = nc.dram_tensor("ipw", [CS, CIN], F32, kind="ExternalInput").ap()
    io["ipb"] = nc.dram_tensor("ipb", [CS], F32, kind="ExternalInput").ap()
    for nm in ("c1w", "c2w"):
        io[nm] = nc.dram_tensor(nm, [NB, CS, CS * 9], F32,
                                kind="ExternalInput").ap()
    for nm in ("c1b", "g1w", "g1b", "c2b", "g2w", "g2b"):
        io[nm] = nc.dram_tensor(nm, [NB, CS], F32, kind="ExternalInput").ap()
    io["ow"] = nc.dram_tensor("ow", [CS, CS], F32, kind="ExternalInput").ap()
    io["ob"] = nc.dram_tensor("ob", [CS], F32, kind="ExternalInput").ap()
    io["w1"] = nc.dram_tensor("w1", [CS + CT, HID], F32,
                              kind="ExternalInput").ap()
    io["b1"] = nc.dram_tensor("b1", [HID], F32, kind="ExternalInput").ap()
    io["w2"] = nc.dram_tensor("w2", [HID, HID], F32, kind="ExternalInput").ap()
    io["b2"] = nc.dram_tensor("b2", [HID], F32, kind="ExternalInput").ap()
    io["w3"] = nc.dram_tensor("w3", [HID, 2], F32, kind="ExternalInput").ap()
    io["b3"] = nc.dram_tensor("b3", [2], F32, kind="ExternalInput").ap()
    io["out"] = nc.dram_tensor("out", [TB, 2], F32, kind="ExternalOutput").ap()
    io["qb"] = nc.dram_tensor("qb", [R, BL, 4 * T], U16).ap()

    with tile.TileContext(nc) as tc:
        with ExitStack() as ctx:
            emit(ctx, tc, io)
    nc.compile()
    return nc


def make_in_maps(inputs: dict) -> list[dict]:
    f = lambda x, d=np.float32: np.ascontiguousarray(np.asarray(x, d))
    weights = {
        "ipw": f(inputs["in_proj_w"]), "ipb": f(inputs["in_proj_b"]),
        "c1w": f(inputs["rb_c1w"]).reshape(NB, CS, CS * 9),
        "c1b": f(inputs["rb_c1b"]), "g1w": f(inputs["rb_g1w"]),
        "g1b": f(inputs["rb_g1b"]),
        "c2w": f(inputs["rb_c2w"]).reshape(NB, CS, CS * 9),
        "c2b": f(inputs["rb_c2b"]), "g2w": f(inputs["rb_g2w"]),
        "g2b": f(inputs["rb_g2b"]),
        "ow": f(inputs["out_w"]), "ob": f(inputs["out_b"]),
        "w1": f(inputs["mlp_w1"]), "b1": f(inputs["mlp_b1"]),
        "w2": f(inputs["mlp_w2"]), "b2": f(inputs["mlp_b2"]),
        "w3": f(inputs["mlp_w3"]), "b3": f(inputs["mlp_b3"]),
    }
    M = f(inputs["M"]).reshape(B_FULL, CM, HIN * WIN)
    x0 = f(inputs["x0_hat_norm"])
    tE = f(inputs["t_embed"])
    maps = []
    for c in range(NCORES):
        sl = slice(c * BL, (c + 1) * BL)
        m = dict(weights)
        m["M"] = np.ascontiguousarray(M[sl])
        m["x0"] = np.ascontiguousarray(x0[sl].reshape(TB, 2))
        m["tE"] = np.ascontiguousarray(tE[sl])
        maps.append(m)
    return maps


_NC_CACHE = {}


def kernel(**inputs) -> np.ndarray:
    if "nc" not in _NC_CACHE:
        _NC_CACHE["nc"] = build_nc()
    nc = _NC_CACHE["nc"]
    in_maps = make_in_maps(inputs)
    res = run_bass_kernel_spmd(nc, in_maps, core_ids=list(range(NCORES)))
    outs = [res.results[c]["out"].reshape(BL, T, 2) for c in range(NCORES)]
    return np.concatenate(outs, axis=0).astype(np.float32)


if __name__ == "__main__":
    nc = build_nc()
    n_inst = sum(len(getattr(f, "instructions", [])) for f in nc.m.functions)
    print(f"built ok, {n_inst} instructions")
